# revision 58
# baseline (speedup 1.0000x reference)
"""Self-contained Trainium2 Bass kernel for GQA MultiHeadAttention with RoPE.

Problem: B=2, S=2048, D=1024, H=16 Q heads, KVH=4 KV heads, head_dim=64,
causal additive mask, f32.

Sharding: tensor-parallel over heads (TP=4: 4 Q heads + 1 KV head per shard)
x data-parallel over batch (DP=2) = 8 NeuronCores. Wo is sharded on its
input dim; the host sums the 4 partial outputs per batch element.

v3: fp8 hi/lo DoubleRow projections (host splits hT and W into e4m3
hi+lo pairs, W pre-scaled by 32 with the 1/32 folded into the rope
tables and Wo), merged/ordered input DMA plan for an early start,
engine-rebalanced rope/norm/evict chains, split tail DMA.
"""

import os
import sys

for _p in ("/opt/trn_rl_repo", "/root/.axon_site/_ro/trn_rl_repo"):
    if os.path.isdir(_p) and _p not in sys.path:
        sys.path.insert(0, _p)

import numpy as np
import ml_dtypes

import concourse.bacc as bacc
import concourse.bass as bass
import concourse.tile as tile
from concourse import mybir
from concourse.bass_utils import run_bass_kernel_spmd

F32 = mybir.dt.float32
BF16 = mybir.dt.bfloat16
F8 = mybir.dt.float8e4
AF = mybir.ActivationFunctionType
DR = mybir.MatmulPerfMode.DoubleRow

H, KVH, HD = 16, 4, 64
B, S, D = 2, 2048, 1024
TP = 4                      # head-parallel ways
SCALE = HD ** -0.5
NEG = -1e9
NT = S // 128               # 16 kv tiles
NQB = S // 512              # 4 q blocks
WSCL = 32.0                 # weight pre-scale (host), folded back via
                            # rope tables (q,k) and Wo (v)


def _env(k, d):
    return os.environ.get(k, d)


def _patch_act_tables():
    """Make Exp resolve only to natural_log_exp_and_others so the
    act-table-load pass emits one load."""
    from concourse.hw_specs import get_activation_tables
    t = get_activation_tables("gen3")
    for name, fns in t.items():
        if name != "natural_log_exp_and_others":
            fns.discard(AF.Exp)
            fns.discard(AF.Ln)


def _build_nc(causal: bool):
    _patch_act_tables()
    nc = bacc.Bacc()

    # hidden states, hi (c 0:8) and lo (c 8:16) concatenated so one DMA
    # fills both per column slice
    htb = nc.declare_dram_parameter("htb", [128, 16, S], F8, isOutput=False)
    # [p, pp, hl, dc*n] (flattened so DMA slices coalesce)
    wq8 = nc.declare_dram_parameter("wq8", [128, 2, 2, 1024], F8,
                                    isOutput=False)
    # [p, hl, dc*n]
    wkv8 = nc.declare_dram_parameter("wkv8", [128, 2, 1024], F8,
                                     isOutput=False)
    # cos/sin (pre-divided by WSCL), duplicated to 128 partitions: [p, 2, S]
    cs2 = nc.declare_dram_parameter("cs2", [128, 2, S], BF16, isOutput=False)
    wo = nc.declare_dram_parameter("wo", [256, D], BF16, isOutput=False)
    # consts blob: [psig | ident | m01 m01] = [128, 512]
    consts = nc.declare_dram_parameter("consts", [128, 512], BF16,
                                       isOutput=False)
    outp = nc.declare_dram_parameter("out", [S, D], BF16, isOutput=True)

    ev_engines = {
        "dve": lambda: nc.vector,
        "pool": lambda: nc.gpsimd,
    }

    with tile.TileContext(nc) as tc:
        with tc.tile_pool(name="hold", bufs=1) as hp:
            # ---- input DMA plan: first-needed first, two dispatch queues
            # (SP HWDGE for the bulk ht stream, Pool SWDGE for the small
            # tables so dispatches overlap) ----
            wsrc = hp.tile([128, 128], BF16, name="wsrc", tag="wsrc")
            nc.vector.memset(wsrc, 0.0)

            ht_b = hp.tile([128, 16, S], F8, name="ht_b", tag="ht_b")
            ht_hi = ht_b[:, 0:8]
            ht_lo = ht_b[:, 8:16]
            wkv_sb = hp.tile([128, 2, 8, 128], F8, name="wkv_sb",
                             tag="wkv_sb")
            nc.sync.dma_start(
                out=wkv_sb[:, 0].rearrange("p a b -> p (a b)"),
                in_=wkv8[:, 0, :])
            nc.sync.dma_start(out=ht_b[:, 0:8, 0:512],
                              in_=htb[:, 0:8, 0:512])
            nc.sync.dma_start(
                out=wkv_sb[:, 1].rearrange("p a b -> p (a b)"),
                in_=wkv8[:, 1, :])

            con_sb = hp.tile([128, 512], BF16, name="con_sb", tag="con_sb")
            nc.gpsimd.dma_start(out=con_sb, in_=consts[:, :])
            psig_sb = con_sb[:, 0:128]
            id_sb = con_sb[:, 128:256]
            m01_sb = con_sb[:, 256:512].rearrange("p (u c) -> p u c", u=2)

            cs_sb = hp.tile([128, 2, S], BF16, name="cs_sb", tag="cs_sb")
            nc.gpsimd.dma_start(out=cs_sb[:, :, 0:512], in_=cs2[:, :, 0:512])
            cosf_sb = cs_sb[:, 0]
            sinf_sb = cs_sb[:, 1]

            wq_sb = hp.tile([128, 2, 2, 8, 128], F8, name="wq_sb",
                            tag="wq_sb")
            nc.sync.dma_start(
                out=wq_sb[:, 0].rearrange("p a b c -> p (a b c)"),
                in_=wq8[:, 0, :, :].rearrange("p a b -> p (a b)"))
            nc.sync.dma_start(out=ht_b[:, 8:16, 0:512],
                              in_=htb[:, 8:16, 0:512])
            nc.sync.dma_start(
                out=wq_sb[:, 1].rearrange("p a b c -> p (a b c)"),
                in_=wq8[:, 1, :, :].rearrange("p a b -> p (a b)"))

            vsm = hp.tile([128, NT, 65], BF16, name="vsm", tag="vsm")
            nc.vector.memset(vsm[:, :, 64:65], 1.0)

            nc.sync.dma_start(out=ht_b[:, 0:8, 512:1024],
                              in_=htb[:, 0:8, 512:1024])
            nc.sync.dma_start(out=ht_b[:, 8:16, 512:1024],
                              in_=htb[:, 8:16, 512:1024])
            nc.sync.dma_start(out=cs_sb[:, :, 512:1024],
                              in_=cs2[:, :, 512:1024])
            nc.sync.dma_start(out=ht_b[:, 0:8, 1024:2048],
                              in_=htb[:, 0:8, 1024:2048])
            nc.sync.dma_start(out=ht_b[:, 8:16, 1024:2048],
                              in_=htb[:, 8:16, 1024:2048])
            nc.sync.dma_start(out=cs_sb[:, :, 1024:2048],
                              in_=cs2[:, :, 1024:2048])

            wo_sb = hp.tile([128, 2, D], BF16, name="wo_sb", tag="wo_sb")
            nc.sync.dma_start(out=wo_sb,
                              in_=wo.rearrange("(c p) n -> p c n", p=128))

            qTs = [hp.tile([128, S], BF16, name=f"qT{p}", tag=f"qT{p}")
                   for p in range(2)]
            qlo = [hp.tile([64, S], BF16, name=f"qlo{p}", tag=f"qlo{p}")
                   for p in range(2)]
            kT = hp.tile([128, S], BF16, name="kTt", tag="kTt")
            ctxTs = [[hp.tile([128, 512], BF16, name=f"ctxT{c}_{q}",
                              tag=f"ctxT{c}_{q}") for q in range(NQB)]
                     for c in range(2)]

            with tc.tile_pool(name="psS", bufs=1, space="PSUM") as psS, \
                 tc.tile_pool(name="psC", bufs=1, space="PSUM") as psC, \
                 tc.tile_pool(name="psD", bufs=1, space="PSUM") as psD, \
                 tc.tile_pool(name="etp", bufs=1) as etp, \
                 tc.tile_pool(name="sbA", bufs=int(_env("KV2_SAB", "5"))) as sbA, \
                 tc.tile_pool(name="sbC", bufs=1) as sbC:

                # PE warmup burst: dependency-free matmuls ramp the PE
                # clock through the DMA-bound lead
                nwarm = int(_env("V3_WARM", "0"))
                for wi in range(nwarm):
                    ps_w = psS.tile([128, 1024], F32, name="ps_w",
                                    tag="ps_s", bufs=2)[:, 0:128]
                    nc.tensor.matmul(ps_w, wsrc, wsrc,
                                     start=True, stop=True)

                # ---------------- Phase A: projections + rope ----------------
                # staged: proj (matmuls+evict) and rope are emitted at
                # different points so PE never head-of-line blocks on the
                # DVE/Pool rope chain
                raws = {}

                def rope_chunk(dst, raw, npart, csl, late=False):
                    # dst = raw*cos + rot(raw)*sin ; raw: bf16 SBUF [npart,n]
                    n = csl.stop - csl.start
                    if not late and _env("V3_RPS", "0") == "1":
                        # attention psS pool is idle through the head phase
                        ps_rot = psS.tile([128, 1024], F32, name="ps_rot",
                                          tag="ps_s", bufs=2)[0:npart, 0:n]
                    else:
                        ps_rot = psD.tile([128, 512], F32, name="ps_rot",
                                          tag="ps_d", bufs=2)[0:npart, 0:n]
                    nc.tensor.matmul(ps_rot, psig_sb[0:npart, 0:npart],
                                     raw, start=True, stop=True)
                    rmc = _env("V3_RMCL" if late else "V3_RMC", "dve")
                    ev_engines[rmc]().tensor_mul(
                        dst, raw, cosf_sb[0:npart, csl])
                    rtmp = sbA.tile([128, 512], BF16, name="rtmp",
                                    tag="rtmp")[:, 0:n]
                    if _env("V3_ROT", "dve") == "act":
                        rotb = sbA.tile([128, 512], BF16, name="rotb",
                                        tag="rotb")[:, 0:n]
                        nc.scalar.copy(rotb[0:npart, :], ps_rot)
                        ev_engines[_env("V3_RMS", "dve")]().tensor_mul(
                            rtmp[0:npart, :], rotb[0:npart, :],
                            sinf_sb[0:npart, csl])
                    else:
                        rms = _env("V3_RMSL" if late else "V3_RMS", "dve")
                        ev_engines[rms]().tensor_mul(
                            rtmp[0:npart, :], ps_rot, sinf_sb[0:npart, csl])
                    rad = _env("V3_RADL" if late else "V3_RAD", "dve")
                    ev_engines[rad]().tensor_add(
                        dst, dst, rtmp[0:npart, :])

                def qkv_term(ps, w, h, csl, ti):
                    for dc in range(4):
                        nc.tensor.matmul(
                            ps,
                            w[:, 2 * dc:2 * dc + 2, :],
                            h[:, 2 * dc:2 * dc + 2, csl],
                            start=(ti == 0 and dc == 0),
                            stop=(ti == 2 and dc == 3),
                            perf_mode=DR)

                def qkv_mms(ps, w_hi, w_lo, csl):
                    # 12 DoubleRow matmuls: (w_hi,h_hi),(w_lo,h_hi),(w_hi,h_lo)
                    for ti, (w, h) in enumerate(
                            [(w_hi, ht_hi), (w_lo, ht_hi), (w_hi, ht_lo)]):
                        qkv_term(ps, w, h, csl, ti)

                def pevict(dst, ps, late=False):
                    ev = _env("V3_QEVL", "dve") if late else _env("V3_QEV", "act")
                    if ev == "act":
                        nc.scalar.copy(dst, ps)
                    else:
                        nc.vector.tensor_copy(dst, ps)

                def q_proj(pp, sc, half=None):
                    csl = (slice(512 * sc, 512 * sc + 512) if half is None
                           else slice(512 * sc + 256 * half,
                                      512 * sc + 256 * half + 256))
                    n = csl.stop - csl.start
                    ps_q = psD.tile([128, 512], F32, name="ps_q",
                                    tag="ps_d", bufs=2)[:, 0:n]
                    qkv_mms(ps_q, wq_sb[:, pp, 0], wq_sb[:, pp, 1], csl)
                    qraw = sbA.tile([128, 512], BF16, name="qraw",
                                    tag="qraw")[:, 0:n]
                    if sc < 2 and _env("V3_QEVQ", "") == "dve":
                        nc.vector.tensor_copy(qraw, ps_q)
                    else:
                        pevict(qraw, ps_q, late=(sc >= 2))
                    raws[("q", pp, csl.start)] = qraw

                def q_rope(pp, sc, half=None, do_qlo=True):
                    csl = (slice(512 * sc, 512 * sc + 512) if half is None
                           else slice(512 * sc + 256 * half,
                                      512 * sc + 256 * half + 256))
                    qraw = raws.pop(("q", pp, csl.start))
                    rope_chunk(qTs[pp][:, csl], qraw, 128, csl, late=(sc >= 2))
                    # odd head's rows to base 0 so all scores matmuls share
                    # one tile_position row base (mixed bases crash HW)
                    if do_qlo:
                        qsl = csl if half is not None else slice(
                            512 * sc, 512 * sc + 512)
                        nc.sync.dma_start(out=qlo[pp][:, qsl],
                                          in_=qTs[pp][64:128, qsl])

                def kv_proj(sc, half=None):
                    csl = (slice(512 * sc, 512 * sc + 512) if half is None
                           else slice(512 * sc + 256 * half,
                                      512 * sc + 256 * half + 256))
                    n = csl.stop - csl.start
                    ps_kv = psD.tile([128, 512], F32, name="ps_kv",
                                     tag="ps_d", bufs=2)[:, 0:n]
                    qkv_mms(ps_kv, wkv_sb[:, 0], wkv_sb[:, 1], csl)
                    kvraw = sbA.tile([128, 512], BF16, name="kvraw",
                                     tag="kvraw")[:, 0:n]
                    pevict(kvraw, ps_kv, late=(sc >= 2))
                    raws[("kv", sc, csl.start)] = kvraw

                def kv_rope(sc, half=None):
                    csl = (slice(512 * sc, 512 * sc + 512) if half is None
                           else slice(512 * sc + 256 * half,
                                      512 * sc + 256 * half + 256))
                    kvraw = raws.pop(("kv", sc, csl.start))
                    # rope on K rows 0:64
                    rope_chunk(kT[0:64, csl], kvraw[0:64, :], 64, csl, late=(sc >= 2))
                    # V rows 64:128: transpose each 128-seq tile into vsm
                    ntt = (csl.stop - csl.start) // 128
                    use_dma = _env("V3_VT", "dma") == "dma" and sc >= 1
                    for tt in range(ntt):
                        ti = csl.start // 128 + tt
                        if use_dma:
                            # XBAR transpose needs a contiguous destination
                            # (strided dst produces wrong output on HW), so
                            # bounce through a temp tile and strided-copy
                            # into vsm via Pool SWDGE
                            vtmp = sbA.tile([128, 64], BF16, name="vtmp",
                                            tag="vtmp", bufs=4)
                            nc.sync.dma_start_transpose(
                                vtmp,
                                kvraw[64:128, 128 * tt:128 * tt + 128])
                            nc.sync.dma_start(out=vsm[:, ti, 0:64],
                                              in_=vtmp)
                            continue
                        ps_v = psD.tile([128, 512], BF16, name="ps_v",
                                        tag="ps_d", bufs=2)[:, 0:64]
                        nc.tensor.matmul(
                            ps_v,
                            kvraw[64:128, 128 * tt:128 * tt + 128],
                            id_sb[64:128, 0:64],
                            start=True, stop=True, is_transpose=True)
                        ev_engines[_env("V3_VEV", "dve")]().tensor_copy(
                            vsm[:, ti, 0:64], ps_v)

                def q_proj_stages(pp, sc):
                    csl = slice(512 * sc, 512 * sc + 512)
                    box = {}

                    def c1():
                        box["ps"] = psD.tile([128, 512], F32, name="ps_q",
                                             tag="ps_d", bufs=2)
                        qkv_term(box["ps"], wq_sb[:, pp, 0], ht_hi, csl, 0)

                    def c2():
                        qkv_term(box["ps"], wq_sb[:, pp, 1], ht_hi, csl, 1)

                    def c3():
                        qkv_term(box["ps"], wq_sb[:, pp, 0], ht_lo, csl, 2)
                        qraw = sbA.tile([128, 512], BF16, name="qraw",
                                        tag="qraw")
                        if sc < 2 and _env("V3_QEVQ", "") == "dve":
                            nc.vector.tensor_copy(qraw, box["ps"])
                        else:
                            pevict(qraw, box["ps"], late=(sc >= 2))
                        raws[("q", pp, csl.start)] = qraw
                    return [c1, c2, c3]

                def kv_proj_stages(sc):
                    csl = slice(512 * sc, 512 * sc + 512)
                    box = {}

                    def c1():
                        box["ps"] = psD.tile([128, 512], F32, name="ps_kv",
                                             tag="ps_d", bufs=2)
                        qkv_term(box["ps"], wkv_sb[:, 0], ht_hi, csl, 0)

                    def c2():
                        qkv_term(box["ps"], wkv_sb[:, 1], ht_hi, csl, 1)

                    def c3():
                        qkv_term(box["ps"], wkv_sb[:, 0], ht_lo, csl, 2)
                        kvraw = sbA.tile([128, 512], BF16, name="kvraw",
                                         tag="kvraw")
                        pevict(kvraw, box["ps"], late=(sc >= 2))
                        raws[("kv", sc, csl.start)] = kvraw
                    return [c1, c2, c3]

                def emit_q_sc(pp, sc):
                    q_proj(pp, sc)
                    q_rope(pp, sc)

                def emit_kv_sc(sc):
                    kv_proj(sc)
                    kv_rope(sc)

                def ost_evict(ost, nb, ps_o, tail=False):
                    # tail D-evicts go to ACT (exp is done by then)
                    if tail and nb == 1:
                        if _env("V3_TEV", "act") == "act":
                            nc.scalar.copy(ost[:, 512 * nb:512 * nb + 512],
                                           ps_o)
                        else:
                            nc.vector.tensor_copy(
                                ost[:, 512 * nb:512 * nb + 512], ps_o)
                        return
                    oev = _env("V3_OEV", "dve")
                    if oev == "act":
                        nc.scalar.copy(ost[:, 512 * nb:512 * nb + 512], ps_o)
                    else:
                        ev_engines[oev]().tensor_copy(
                            ost[:, 512 * nb:512 * nb + 512], ps_o)

                def emit_d_qt(qb, qt):
                    ost = sbC.tile([128, 1024], BF16, name="ost", tag="ost",
                                   bufs=int(_env("KV2_OSTB", "8")))
                    for nb in range(2):
                        ps_o = psD.tile([128, 512], F32, name="ps_o",
                                        tag="ps_d", bufs=2)
                        for c in range(2):
                            nc.tensor.matmul(
                                ps_o,
                                ctxTs[c][qb][:, 128 * qt:128 * qt + 128],
                                wo_sb[:, c, 512 * nb:512 * nb + 512],
                                start=(c == 0), stop=(c == 1))
                        ost_evict(ost, nb, ps_o)
                    row = 512 * qb + 128 * qt
                    nc.sync.dma_start(out=outp[row:row + 128, :], in_=ost)

                tail_ps_n = [0]

                def emit_phase_d(qb, qts, tail=False):
                    for qt in qts:
                        ost = sbC.tile([128, 1024], BF16, name="ost",
                                       tag="ost",
                                       bufs=int(_env("KV2_OSTB", "8")))
                        split = (tail and (
                            (qb == 3 and qt == qts[-1]) or
                            _env("V3_TAS", "0") == "1") and
                            _env("V3_LS", "0") == "1")
                        for nb in range(2):
                            if tail and _env("V3_TPS", "1") == "1":
                                # attention psum pools are free by the tail:
                                # rotate D psums through them so evicts
                                # never gate the next matmul. psC is held
                                # by the final norm chains - delay its use.
                                i = tail_ps_n[0]
                                tail_ps_n[0] += 1
                                tv = _env("V3_TPSV", "a")
                                if tv == "nc":
                                    r = [0, 2][i % 2]
                                elif tv == "c2":
                                    r = [0, 2, 0, 2, 1][i % 5] if i >= 2 \
                                        else [0, 2][i]
                                else:
                                    r = [0, 2, 0, 2][i] if i < 4 else (i % 3)
                                if r == 0:
                                    ps_o = psS.tile([128, 1024], F32,
                                                    name="ps_o", tag="ps_s",
                                                    bufs=2)[:, 0:512]
                                elif r == 1:
                                    ps_o = psC.tile([128, 512], F32,
                                                    name="ps_o",
                                                    tag="ps_ctx", bufs=2)
                                else:
                                    ps_o = psD.tile([128, 512], F32,
                                                    name="ps_o", tag="ps_d",
                                                    bufs=2)
                            else:
                                ps_o = psD.tile([128, 512], F32, name="ps_o",
                                                tag="ps_d", bufs=2)
                            for c in range(2):
                                ct = ctxTs[c][qb]
                                col = 128 * qt
                                nc.tensor.matmul(
                                    ps_o,
                                    ct[:, col:col + 128],
                                    wo_sb[:, c, 512 * nb:512 * nb + 512],
                                    start=(c == 0), stop=(c == 1))
                            ost_evict(ost, nb, ps_o, tail=tail)
                            if split:
                                # pipeline the last tile's DMA per-half so
                                # the final transfer is short; first half
                                # goes out via Pool SWDGE so the final
                                # HWDGE dispatch isn't queued behind it
                                row = 512 * qb + 128 * qt
                                last = qb == 3 and qt == qts[-1]
                                eng = (nc.gpsimd if nb == 0 and last and
                                       _env("V3_TSP", "1") == "1"
                                       else nc.sync)
                                eng.dma_start(
                                    out=outp[row:row + 128,
                                             512 * nb:512 * nb + 512],
                                    in_=ost[:, 512 * nb:512 * nb + 512])
                        if not split:
                            row = 512 * qb + 128 * qt
                            eng = (nc.gpsimd if tail and qb == 3 and
                                   qt == 2 and _env("V3_T2P", "0") == "1"
                                   else nc.sync)
                            eng.dma_start(
                                out=outp[row:row + 128, :],
                                in_=ost)

                def norm_tail(h, qb, ps_ctx):
                    # normalization; split mode evicts the unnormalized ctx
                    # immediately (ACT) so the psC slot frees in ~0.7us
                    # instead of holding through the recip/bcast/mul chain
                    rs = sbC.tile([1, 512], F32, name="rs", tag="rs",
                                  bufs=int(_env("KV2_RSB", "3")))
                    nc.vector.reciprocal(rs, ps_ctx[64:65, :])
                    rb = sbC.tile([64, 512], F32, name="rb", tag="rb",
                                  bufs=int(_env("KV2_RBB", "6")))
                    nc.gpsimd.partition_broadcast(rb, rs, channels=64)
                    c = h % 2
                    nrm = ev_engines[_env("V3_NRM", "dve")]()
                    split = _env("V3_NSPLIT", "0") == "1"
                    if split:
                        cu = sbC.tile([64, 512], BF16, name="cu", tag="cu",
                                      bufs=int(_env("V3_CUB", "3")))
                        if _env("V3_CUE", "act") == "act":
                            nc.scalar.copy(cu, ps_ctx[0:64, :])
                        else:
                            nc.vector.tensor_copy(cu, ps_ctx[0:64, :])
                        csrc = cu
                    else:
                        csrc = ps_ctx[0:64, :]
                    if h < 2:
                        nrm.tensor_mul(ctxTs[c][qb][0:64, :], csrc, rb)
                    else:
                        csh = sbC.tile([64, 512], BF16, name="csh",
                                       tag="csh",
                                       bufs=int(_env("KV2_CSB", "2")))
                        nrm.tensor_mul(csh, csrc, rb)
                        nc.sync.dma_start(out=ctxTs[c][qb][64:128, :],
                                          in_=csh)

                def emit_bc2(qb, ha, hb, mid=(), fillers=()):
                    # two heads sharing one diag-B psum+exp unit: head X's
                    # diag tiles j2,j3 at et cols [0:384], head Y's at
                    # [384:768] -> one [128,768] exp instead of two [128,384]
                    qsl = slice(512 * qb, 512 * qb + 512)
                    nfull = 4 * qb
                    nki = 4 * qb + 4
                    shared = {}

                    # region layout keeps every matmul output inside one
                    # psum bank: ha-j2 [0:256], ha-j3 [256:384],
                    # hb-j3 [384:512], hb-j2 [512:768]
                    REG = {(0, 2): 0, (0, 3): 256, (1, 3): 384, (1, 2): 512}

                    def s_B():
                        ps_sB = psS.tile([128, 1024], F32, name="ps_s",
                                         tag="ps_s", bufs=2)
                        for idx, h in ((0, ha), (1, hb)):
                            pp = h // 2
                            qsrc = qTs[pp] if h % 2 == 0 else qlo[pp]
                            for j in (2, 3):
                                cl = REG[(idx, j)]
                                ki = 4 * qb + j
                                span = 512 - 128 * j
                                nc.tensor.matmul(
                                    ps_sB[:, cl:cl + span],
                                    kT[0:64, 128 * ki:128 * ki + 128],
                                    qsrc[0:64,
                                         512 * qb + 128 * j:512 * (qb + 1)],
                                    start=True, stop=True)
                        shared["ps"] = ps_sB

                    def e_B():
                        etB = etp.tile([128, 1024], BF16, name="et", tag="et",
                                       bufs=int(_env("KV2_ETB", "34")))
                        nc.scalar.activation(etB[:, 0:768],
                                             shared["ps"][:, 0:768],
                                             AF.Exp, scale=SCALE)
                        # triangles: ha at 0 & 256 (stride 256), hb at 384 &
                        # 512 (stride 128)
                        tri_eng = ev_engines[_env("V3_TRI", "dve")]()
                        tri_a = etB[:, 0:512].rearrange(
                            "p (u c) -> p u c", u=2)[:, :, 0:128]
                        tri_eng.tensor_mul(tri_a, tri_a, m01_sb)
                        tri_b = etB[:, 384:640].rearrange(
                            "p (u c) -> p u c", u=2)
                        tri_eng.tensor_mul(tri_b, tri_b, m01_sb)
                        shared["et"] = etB

                    def ctx_B(idx, ps_ctx, vsl):
                        for j in (2, 3):
                            cl = REG[(idx, j)]
                            ki = 4 * qb + j
                            span = 512 - 128 * j
                            nc.tensor.matmul(
                                ps_ctx[:, 128 * j:512],
                                vsm[:, ki, vsl],
                                shared["et"][:, cl:cl + span],
                                start=(ki == 0), stop=(ki == nki - 1))

                    def head_part(idx, h):
                        pp = h // 2
                        qsrc = qTs[pp] if h % 2 == 0 else qlo[pp]
                        ps_ctx = psC.tile([128, 512], F32, name="ps_ctx",
                                          tag="ps_ctx", bufs=2)[0:65, :]
                        vsl = slice(0, 65)
                        units = []

                        def mk_pair(kp):
                            box = {}

                            def s():
                                ps_s = psS.tile([128, 1024], F32, name="ps_s",
                                                tag="ps_s", bufs=2)
                                for jj in range(2):
                                    ki = kp + jj
                                    nc.tensor.matmul(
                                        ps_s[:, 512 * jj:512 * jj + 512],
                                        kT[0:64, 128 * ki:128 * ki + 128],
                                        qsrc[0:64, qsl],
                                        start=True, stop=True)
                                box["ps"] = ps_s

                            def ec():
                                et = etp.tile([128, 1024], BF16, name="et",
                                              tag="et",
                                              bufs=int(_env("KV2_ETB", "34")))
                                nc.scalar.activation(et, box["ps"], AF.Exp,
                                                     scale=SCALE)
                                for jj in range(2):
                                    ki = kp + jj
                                    nc.tensor.matmul(
                                        ps_ctx,
                                        vsm[:, ki, vsl],
                                        et[:, 512 * jj:512 * jj + 512],
                                        start=(ki == 0), stop=(ki == nki - 1))
                            return (s, ec)

                        def mk_diag_a():
                            box = {}

                            def s():
                                ps_s = psS.tile([128, 1024], F32, name="ps_s",
                                                tag="ps_s", bufs=2)
                                for j, cl in ((0, 0), (1, 512)):
                                    ki = 4 * qb + j
                                    span = 512 - 128 * j
                                    nc.tensor.matmul(
                                        ps_s[:, cl:cl + span],
                                        kT[0:64, 128 * ki:128 * ki + 128],
                                        qsrc[0:64,
                                             512 * qb + 128 * j:512 * (qb + 1)],
                                        start=True, stop=True)
                                box["ps"] = ps_s

                            def ec():
                                et = etp.tile([128, 1024], BF16, name="et",
                                              tag="et",
                                              bufs=int(_env("KV2_ETB", "34")))
                                nc.scalar.activation(et[:, 0:896],
                                                     box["ps"][:, 0:896],
                                                     AF.Exp, scale=SCALE)
                                tri = et[:, 0:1024].rearrange(
                                    "p (u c) -> p u c", u=2)[:, :, 0:128]
                                ev_engines[_env("V3_TRI", "dve")]().tensor_mul(
                                    tri, tri, m01_sb)
                                for j, cl in ((0, 0), (1, 512)):
                                    ki = 4 * qb + j
                                    span = 512 - 128 * j
                                    nc.tensor.matmul(
                                        ps_ctx[:, 128 * j:512],
                                        vsm[:, ki, vsl],
                                        et[:, cl:cl + span],
                                        start=(ki == 0), stop=(ki == nki - 1))
                            return (s, ec)

                        for kp in range(0, nfull, 2):
                            units.append(mk_pair(kp))
                        units.append(mk_diag_a())
                        if idx == 0:
                            units.append((s_B, e_B))
                        else:
                            units.append((lambda: None,
                                          lambda: ctx_B(1, ps_ctx, vsl)))
                        units[0][0]()
                        fcad = int(_env("V3_FCAD", "2"))
                        for i in range(len(units)):
                            if i + 1 < len(units):
                                units[i + 1][0]()
                            units[i][1]()
                            if fil and i % fcad == fcad - 1:
                                fil.pop(0)()
                        if idx == 0:
                            ctx_B(0, ps_ctx, vsl)

                        norm_tail(h, qb, ps_ctx)

                    fil = list(fillers)
                    head_part(0, ha)
                    for m in mid:
                        m()
                    head_part(1, hb)
                    for f in fil:
                        f()

                def emit_bc(qb, h, fillers=()):
                    # attention for one (q block, head): scores, exp, ctx,
                    # normalization fused into the evict
                    pp = h // 2
                    qsrc = qTs[pp] if h % 2 == 0 else qlo[pp]
                    qsl = slice(512 * qb, 512 * qb + 512)
                    ps_ctx = psC.tile([128, 512], F32, name="ps_ctx",
                                      tag="ps_ctx", bufs=2)[0:65, :]
                    vsl = slice(0, 65)
                    nfull = (4 * qb) if causal else NT
                    nki = (4 * qb + 4) if causal else NT
                    units = []

                    def mk_pair(kp):
                        # full pair unit: kv tiles (kp, kp+1)
                        box = {}

                        def s():
                            ps_s = psS.tile([128, 1024], F32, name="ps_s",
                                            tag="ps_s", bufs=2)
                            for jj in range(2):
                                ki = kp + jj
                                nc.tensor.matmul(
                                    ps_s[:, 512 * jj:512 * jj + 512],
                                    kT[0:64, 128 * ki:128 * ki + 128],
                                    qsrc[0:64, qsl],
                                    start=True, stop=True)
                            box["ps"] = ps_s

                        def ec():
                            et = etp.tile([128, 1024], BF16, name="et",
                                          tag="et",
                                          bufs=int(_env("KV2_ETB", "34")))
                            nc.scalar.activation(et, box["ps"], AF.Exp,
                                                 scale=SCALE)
                            for jj in range(2):
                                ki = kp + jj
                                nc.tensor.matmul(
                                    ps_ctx,
                                    vsm[:, ki, vsl],
                                    et[:, 512 * jj:512 * jj + 512],
                                    start=(ki == 0), stop=(ki == nki - 1))
                        return (s, ec)

                    def mk_diag(du):
                        # diag unit du=0: tiles j=0,1 at cols 0 / 512
                        #           du=1: tiles j=2,3 at cols 0 / 256
                        box = {}
                        js = (0, 1) if du == 0 else (2, 3)
                        cols = (0, 512) if du == 0 else (0, 256)
                        wid = 896 if du == 0 else 384

                        def s():
                            ps_s = psS.tile([128, 1024], F32, name="ps_s",
                                            tag="ps_s", bufs=2)
                            for j, cl in zip(js, cols):
                                ki = 4 * qb + j
                                span = 512 - 128 * j
                                nc.tensor.matmul(
                                    ps_s[:, cl:cl + span],
                                    kT[0:64, 128 * ki:128 * ki + 128],
                                    qsrc[0:64,
                                         512 * qb + 128 * j:512 * (qb + 1)],
                                    start=True, stop=True)
                            box["ps"] = ps_s

                        def ec():
                            et = etp.tile([128, 1024], BF16, name="et",
                                          tag="et",
                                          bufs=int(_env("KV2_ETB", "34")))
                            nc.scalar.activation(et[:, 0:wid],
                                                 box["ps"][:, 0:wid],
                                                 AF.Exp, scale=SCALE)
                            # zero the causal triangles: both triangles in
                            # this unit are 128 wide, cols[1] apart
                            tri = et[:, 0:2 * cols[1]].rearrange(
                                "p (u c) -> p u c", u=2)[:, :, 0:128]
                            nc.vector.tensor_mul(tri, tri, m01_sb)
                            for j, cl in zip(js, cols):
                                ki = 4 * qb + j
                                span = 512 - 128 * j
                                nc.tensor.matmul(
                                    ps_ctx[:, 128 * j:512],
                                    vsm[:, ki, vsl],
                                    et[:, cl:cl + span],
                                    start=(ki == 0), stop=(ki == nki - 1))
                        return (s, ec)

                    for kp in range(0, nfull, 2):
                        units.append(mk_pair(kp))
                    if causal:
                        units.append(mk_diag(0))
                        units.append(mk_diag(1))
                    if units:
                        units[0][0]()
                    fi = list(fillers)
                    for i in range(len(units)):
                        if i + 1 < len(units):
                            units[i + 1][0]()
                        units[i][1]()
                        # interleave ready phase-D tiles into the unit
                        # sequence so they fill PE gaps while ACT paces
                        if fi and i % 2 == 1:
                            fi.pop(0)()
                    for f in fi:
                        f()

                    norm_tail(h, qb, ps_ctx)

                # ---- global emission order ----
                # attention on q-block qb needs rope chunks 0..qb of q/k;
                # A-phase chunks are threaded through the bc groups so the
                # rope for chunk sc overlaps attention on earlier blocks
                gv = _env("KV2_GVAR", "7")
                ho = [int(x) for x in _env("KV2_HORD", "0123")]
                if gv == "7":
                    # pipelined head, staged proj/rope; first bc2 of each
                    # block pairs the EVEN heads (0,2) so no qlo DMA is on
                    # the critical path
                    if _env("V3_HALF", "0") == "1":
                        kv_proj(0, half=0)
                        kv_proj(0, half=1)
                        q_proj(0, 0, half=0)
                        kv_rope(0, half=0)
                        q_proj(0, 0, half=1)
                        kv_rope(0, half=1)
                        q_rope(0, 0, half=0)
                        q_proj(1, 0, half=0)
                        q_rope(0, 0, half=1)
                        q_proj(1, 0, half=1)
                        q_rope(1, 0, half=0)
                        q_rope(1, 0, half=1)
                    else:
                        hv = _env("V3_HV", "5")
                        if hv == "7":
                            # like 6 but kT's rope follows the kv evict
                            # immediately so the K chain isn't delayed
                            kvs = kv_proj_stages(0)
                            qs = q_proj_stages(0, 0)
                            kvs[0]()
                            kvs[1]()
                            qs[0]()
                            qs[1]()
                            kvs[2]()
                            kv_rope(0)
                            qs[2]()
                            q_rope(0, 0)
                        elif hv == "6":
                            # interleave kv/q projection terms so q's
                            # hi-terms run while kv's lo-terms wait on htl
                            kvs = kv_proj_stages(0)
                            qs = q_proj_stages(0, 0)
                            kvs[0]()
                            kvs[1]()
                            qs[0]()
                            qs[1]()
                            kvs[2]()
                            qs[2]()
                            kv_rope(0)
                            q_rope(0, 0)
                        else:
                            kv_proj(0)
                            q_proj(0, 0)
                            kv_rope(0)
                            if hv not in ("5", "6"):
                                q_proj(1, 0)
                                q_rope(0, 0)
                                q_rope(1, 0)
                            else:
                                q_rope(0, 0)
                    hpair = _env("V3_PAIR", "02")
                    if hpair == "02":
                        pairs = ((0, 2), (1, 3))
                    else:
                        pairs = ((0, 1), (2, 3))
                    use_fil = _env("V3_AFIL", "0") == "1"
                    dv15 = _env("KV2_DVAR", "13") == "15"
                    for qb in range(3):
                        fil2 = ([(lambda qt=qt: emit_d_qt(0, qt))
                                 for qt in range(4)]
                                if (dv15 and qb == 2) else [])
                        if qb == 0 and _env("V3_HV", "5") in ("5", "6", "7"):
                            # head 2 only needs qT1 at the second
                            # head_part: fold q1's proj+rope into mid so
                            # attention starts as soon as qT0 is roped
                            if _env("V3_HV5M", "a") == "b":
                                emit_bc2(0, *pairs[0],
                                         mid=[lambda: kv_proj(1),
                                              lambda: q_proj(1, 0),
                                              lambda: q_rope(1, 0),
                                              lambda: q_proj(0, 1)])
                            else:
                                emit_bc2(0, *pairs[0],
                                         mid=[lambda: q_proj(1, 0),
                                              lambda: kv_proj(1),
                                              lambda: q_rope(1, 0),
                                              lambda: q_proj(0, 1)])
                            emit_bc2(0, *pairs[1],
                                     mid=[lambda: kv_rope(1),
                                          lambda: q_rope(0, 1),
                                          lambda: q_proj(1, 1)],
                                     fillers=fil2)
                            q_rope(1, 1)
                            continue
                        if use_fil:
                            emit_bc2(qb, *pairs[0],
                                     fillers=(kv_proj_stages(qb + 1)
                                              + q_proj_stages(0, qb + 1)))
                            emit_bc2(qb, *pairs[1],
                                     mid=[lambda qb=qb: kv_rope(qb + 1),
                                          lambda qb=qb: q_rope(0, qb + 1)],
                                     fillers=q_proj_stages(1, qb + 1) + fil2)
                        else:
                            mv = _env("V3_MIDV", "A")
                            if mv == "E":
                                emit_bc2(qb, *pairs[0],
                                         mid=[lambda qb=qb: kv_proj(qb + 1),
                                              lambda qb=qb: q_proj(0, qb + 1),
                                              lambda qb=qb: kv_rope(qb + 1)])
                                emit_bc2(qb, *pairs[1],
                                         mid=[lambda qb=qb: q_rope(0, qb + 1),
                                              lambda qb=qb: q_proj(1, qb + 1)],
                                         fillers=fil2)
                            elif mv == "F":
                                emit_bc2(qb, *pairs[0],
                                         mid=[lambda qb=qb: kv_proj(qb + 1),
                                              lambda qb=qb: q_proj(0, qb + 1)])
                                emit_bc2(qb, *pairs[1],
                                         mid=[lambda qb=qb: q_proj(1, qb + 1),
                                              lambda qb=qb: kv_rope(qb + 1),
                                              lambda qb=qb: q_rope(0, qb + 1)],
                                         fillers=fil2)
                            elif mv == "B":
                                emit_bc2(qb, *pairs[0],
                                         mid=[lambda qb=qb: kv_proj(qb + 1),
                                              lambda qb=qb: q_proj(0, qb + 1),
                                              lambda qb=qb: q_proj(1, qb + 1)])
                                emit_bc2(qb, *pairs[1],
                                         mid=[lambda qb=qb: kv_rope(qb + 1),
                                              lambda qb=qb: q_rope(0, qb + 1)],
                                         fillers=fil2)
                            elif mv == "D":
                                emit_bc2(qb, *pairs[0],
                                         mid=[lambda qb=qb: kv_proj(qb + 1),
                                              lambda qb=qb: q_proj(0, qb + 1),
                                              lambda qb=qb: q_proj(1, qb + 1)])
                                emit_bc2(qb, *pairs[1],
                                         mid=[lambda qb=qb: kv_rope(qb + 1),
                                              lambda qb=qb: q_rope(0, qb + 1),
                                              lambda qb=qb: q_rope(1, qb + 1)],
                                         fillers=fil2)
                            else:
                                emit_bc2(qb, *pairs[0],
                                         mid=[lambda qb=qb: kv_proj(qb + 1),
                                              lambda qb=qb: q_proj(0, qb + 1)])
                                emit_bc2(qb, *pairs[1],
                                         mid=[lambda qb=qb: kv_rope(qb + 1),
                                              lambda qb=qb: q_rope(0, qb + 1),
                                              lambda qb=qb: q_proj(1, qb + 1)],
                                         fillers=fil2)
                        if _env("V3_MIDV", "A") != "D":
                            q_rope(1, qb + 1)
                else:
                    emit_kv_sc(0)
                    emit_q_sc(0, 0)
                    emit_q_sc(1, 0)
                if gv == "7":
                    pass
                elif gv == "6":
                    for qb in range(3):
                        emit_bc2(qb, 0, 1,
                                 mid=[lambda qb=qb: emit_kv_sc(qb + 1)])
                        emit_bc2(qb, 2, 3,
                                 mid=[lambda qb=qb: emit_q_sc(0, qb + 1)])
                        emit_q_sc(1, qb + 1)
                elif gv == "5":
                    for qb in range(3):
                        emit_bc2(qb, 0, 1)
                        emit_kv_sc(qb + 1)
                        emit_bc2(qb, 2, 3,
                                 mid=[lambda qb=qb: emit_q_sc(0, qb + 1)])
                        emit_q_sc(1, qb + 1)
                elif gv == "8":
                    for qb in range(3):
                        emit_bc2(qb, 0, 1,
                                 mid=[lambda qb=qb: emit_kv_sc(qb + 1)])
                        emit_bc2(qb, 2, 3,
                                 mid=[lambda qb=qb: emit_q_sc(0, qb + 1),
                                      lambda qb=qb: emit_q_sc(1, qb + 1)])
                else:
                    for qb in range(3):
                        emit_bc(qb, ho[0])
                        emit_bc(qb, ho[1])
                        emit_bc(qb, ho[2])
                        emit_kv_sc(qb + 1)
                        emit_bc(qb, ho[3])
                        emit_q_sc(0, qb + 1)
                        emit_q_sc(1, qb + 1)
                dv = int(_env("KV2_DVAR", "13"))
                tp = _env("V3_TPAIR", "23_01")
                tpairs = {"13_20": ((1, 3), (2, 0)), "23_10": ((2, 3), (1, 0)),
                          "23_01": ((2, 3), (0, 1)), "31_20": ((3, 1), (2, 0))}[tp]
                if dv == 11:
                    emit_bc2(3, *tpairs[0])
                    emit_phase_d(0, [0, 1, 2, 3])
                    emit_bc2(3, *tpairs[1])
                    emit_phase_d(1, [0, 1, 2, 3])
                    emit_phase_d(2, [0, 1, 2, 3], tail=True)
                    emit_phase_d(3, [0, 1, 2, 3], tail=True)
                elif dv == 10:
                    emit_bc2(3, 1, 2)
                    emit_phase_d(0, [0, 1, 2, 3])
                    emit_bc2(3, 3, 0)
                    emit_phase_d(1, [0, 1, 2, 3])
                    emit_phase_d(2, [0, 1, 2, 3], tail=True)
                    emit_phase_d(3, [0, 1, 2, 3], tail=True)
                elif dv == 15:
                    emit_bc2(3, *tpairs[0],
                             fillers=[(lambda qt=qt: emit_d_qt(1, qt))
                                      for qt in range(4)])
                    emit_bc2(3, *tpairs[1],
                             fillers=[(lambda qt=qt: emit_d_qt(2, qt))
                                      for qt in range(4)])
                    emit_phase_d(3, [0, 1, 2, 3], tail=True)
                elif dv == 13:
                    emit_bc2(3, *tpairs[0],
                             fillers=[(lambda qt=qt: emit_d_qt(0, qt))
                                      for qt in range(4)])
                    emit_bc2(3, *tpairs[1],
                             fillers=[(lambda qt=qt: emit_d_qt(1, qt))
                                      for qt in range(4)]
                             + [(lambda qt=qt: emit_d_qt(2, qt))
                                for qt in range(2)])
                    emit_phase_d(2, [2, 3], tail=True)
                    emit_phase_d(3, [0, 1, 2, 3], tail=True)
                elif dv == 18:
                    emit_bc2(3, *tpairs[0],
                             fillers=[(lambda qt=qt: emit_d_qt(0, qt))
                                      for qt in range(3)])
                    emit_d_qt(0, 3)
                    emit_bc2(3, *tpairs[1],
                             fillers=[(lambda qt=qt: emit_d_qt(1, qt))
                                      for qt in range(4)]
                             + [(lambda qt=qt: emit_d_qt(2, qt))
                                for qt in range(2)])
                    emit_phase_d(2, [2, 3], tail=True)
                    emit_phase_d(3, [0, 1, 2, 3], tail=True)
                elif dv == 17:
                    emit_bc2(3, *tpairs[0],
                             fillers=[(lambda qt=qt: emit_d_qt(0, qt))
                                      for qt in range(4)])
                    emit_bc2(3, *tpairs[1],
                             fillers=[(lambda qt=qt: emit_d_qt(1, qt))
                                      for qt in range(4)]
                             + [lambda: emit_d_qt(2, 0)])
                    emit_phase_d(2, [1, 2, 3], tail=True)
                    emit_phase_d(3, [0, 1, 2, 3], tail=True)
                elif dv == 16:
                    emit_bc2(3, *tpairs[0],
                             fillers=[(lambda qt=qt: emit_d_qt(0, qt))
                                      for qt in range(4)])
                    emit_bc2(3, *tpairs[1],
                             fillers=[(lambda qt=qt: emit_d_qt(1, qt))
                                      for qt in range(4)])
                    emit_phase_d(2, [0, 1, 2, 3], tail=True)
                    emit_phase_d(3, [0, 1, 2, 3], tail=True)
                elif dv == 14:
                    emit_bc2(3, *tpairs[0],
                             fillers=[(lambda qt=qt: emit_d_qt(0, qt))
                                      for qt in range(4)])
                    emit_bc2(3, *tpairs[1],
                             fillers=[(lambda qt=qt: emit_d_qt(1, qt))
                                      for qt in range(4)]
                             + [(lambda qt=qt: emit_d_qt(2, qt))
                                for qt in range(4)])
                    emit_phase_d(3, [0, 1, 2, 3], tail=True)
                elif dv == 12:
                    emit_bc2(3, 1, 2,
                             mid=[lambda: emit_phase_d(0, [0, 1])])
                    emit_phase_d(0, [2, 3])
                    emit_bc2(3, 3, 0,
                             mid=[lambda: emit_phase_d(1, [0, 1])])
                    emit_phase_d(1, [2, 3])
                    emit_phase_d(2, [0, 1, 2, 3], tail=True)
                    emit_phase_d(3, [0, 1, 2, 3], tail=True)
                else:
                    emit_bc(3, 0)
                    emit_bc(3, 1)
                    emit_phase_d(0, [0, 1, 2, 3])
                    emit_bc(3, 2)
                    emit_phase_d(1, [0, 1, 2, 3])
                    emit_bc(3, 3)
                    emit_phase_d(2, [0, 1, 2, 3])
                    emit_phase_d(3, [0, 1, 2, 3], tail=True)

    nc.compile()
    return nc


_NC_CACHE = {}


def _get_nc(causal: bool):
    if causal not in _NC_CACHE:
        _NC_CACHE[causal] = _build_nc(causal)
    return _NC_CACHE[causal]


def _host_consts():
    p = np.zeros((128, 128), np.float32)
    idx = np.arange(0, 128, 2)
    p[idx, idx + 1] = -1.0
    p[idx + 1, idx] = 1.0
    psigT = np.ascontiguousarray(p.T)
    ident = np.zeros((128, 128), np.float32)
    ident[64:128, 0:64] = np.eye(64, dtype=np.float32)
    m01 = (np.arange(128)[None, :] >= np.arange(128)[:, None]).astype(np.float32)
    return np.concatenate([psigT, ident, m01, m01], axis=1)


def _numpy_reference(hidden_states, cos, sin, attention_mask, Wq, Wk, Wv, Wo):
    """Generic-mask fallback, pure numpy port of the reference."""
    GROUPS = H // KVH

    def rope(x, c, s):
        c = c[:, None, :, :]
        s = s[:, None, :, :]
        x1, x2 = x[..., ::2], x[..., 1::2]
        xr = np.stack([x1 * c - x2 * s, x1 * s + x2 * c], axis=-1)
        return xr.reshape(x.shape)

    b, sq, d = hidden_states.shape
    q = (hidden_states @ Wq).reshape(b, sq, H, HD).transpose(0, 2, 1, 3)
    k = (hidden_states @ Wk).reshape(b, sq, KVH, HD).transpose(0, 2, 1, 3)
    v = (hidden_states @ Wv).reshape(b, sq, KVH, HD).transpose(0, 2, 1, 3)
    q = rope(q, cos, sin)
    k = rope(k, cos, sin)
    k = np.repeat(k, GROUPS, axis=1)
    v = np.repeat(v, GROUPS, axis=1)
    out = np.zeros((b, sq, d), np.float32)
    for bi in range(b):
        for hi in range(H):
            sc = (q[bi, hi] @ k[bi, hi].T) * SCALE + attention_mask[0, 0]
            sc = sc - sc.max(axis=-1, keepdims=True)
            e = np.exp(sc)
            pr = e / e.sum(axis=-1, keepdims=True)
            ctx = pr @ v[bi, hi]
            out[bi] += ctx @ Wo[hi * HD:(hi + 1) * HD]
    return out


def _split8(x):
    f8 = ml_dtypes.float8_e4m3
    hi = x.astype(f8)
    lo = (x - hi.astype(np.float32)).astype(f8)
    return hi, lo


def kernel(**inputs) -> np.ndarray:
    hs = np.asarray(inputs["hidden_states"], np.float32)
    cos = np.asarray(inputs["cos"], np.float32)
    sin = np.asarray(inputs["sin"], np.float32)
    mask = np.asarray(inputs["attention_mask"], np.float32)
    Wq = np.asarray(inputs["Wq"], np.float32)
    Wk = np.asarray(inputs["Wk"], np.float32)
    Wv = np.asarray(inputs["Wv"], np.float32)
    Wo = np.asarray(inputs["Wo"], np.float32)

    m = mask.reshape(S, S)
    tril = np.tril(np.ones((S, S), dtype=bool))
    causal_ref = np.where(tril, np.float32(0.0), np.float32(NEG))
    if np.array_equal(m, causal_ref):
        causal = True
    elif not m.any():
        causal = False
    else:
        return _numpy_reference(hs, cos, sin, mask, Wq, Wk, Wv, Wo)

    nc = _get_nc(causal)
    consts = _host_consts()
    chan_half = (np.arange(64) // 2)

    bf = ml_dtypes.bfloat16
    in_maps = []
    for core in range(8):
        b, t = core // TP, core % TP
        # hidden states, swizzled [p, c, s], fp8 hi/lo
        hsw = np.ascontiguousarray(
            hs[b].T.reshape(8, 128, S).transpose(1, 0, 2))
        h_hi, h_lo = _split8(hsw)
        h_b = np.ascontiguousarray(np.concatenate([h_hi, h_lo], axis=1))

        # Wq shard [1024, 256] -> [p, pp, hl, dc, n]
        wq_s = Wq[:, t * 256:(t + 1) * 256] * WSCL
        wq_sw = wq_s.reshape(8, 128, 256).transpose(1, 0, 2)  # [p, dc, 256]
        wq_pp = np.stack([wq_sw[:, :, 0:128], wq_sw[:, :, 128:256]],
                         axis=1)                              # [p, pp, dc, n]
        wq_hi, wq_lo = _split8(wq_pp)
        wq8 = np.ascontiguousarray(
            np.stack([wq_hi, wq_lo], axis=2)).reshape(128, 2, 2, 1024)

        # Wk|Wv shard [1024, 128] -> [p, hl, dc, n]
        wkv_s = np.concatenate([Wk[:, t * 64:(t + 1) * 64],
                                Wv[:, t * 64:(t + 1) * 64]], axis=1) * WSCL
        wkv_sw = wkv_s.reshape(8, 128, 128).transpose(1, 0, 2)
        wkv_hi, wkv_lo = _split8(wkv_sw)
        wkv8 = np.ascontiguousarray(
            np.stack([wkv_hi, wkv_lo], axis=1)).reshape(128, 2, 1024)

        # rope tables: [64, S] expanded from half tables, / WSCL, dup to 128
        cs64v = cos[b].T[chan_half, :] / WSCL
        sn64v = sin[b].T[chan_half, :] / WSCL
        cs2 = np.ascontiguousarray(np.stack(
            [np.concatenate([cs64v, cs64v], axis=0),
             np.concatenate([sn64v, sn64v], axis=0)], axis=1)).astype(bf)

        wo_s = Wo[t * 256:(t + 1) * 256] / WSCL
        # ctxT channel order per chunk: c0 = [h0|h2], c1 = [h1|h3]
        wo_p = np.ascontiguousarray(
            np.concatenate([wo_s[0:64], wo_s[128:192],
                            wo_s[64:128], wo_s[192:256]], axis=0)).astype(bf)
        in_maps.append({
            "htb": h_b, "wq8": wq8, "wkv8": wkv8,
            "cs2": cs2, "wo": wo_p,
            "consts": consts.astype(bf),
        })

    res = run_bass_kernel_spmd(nc, in_maps, core_ids=list(range(8)))
    out = np.zeros((B, S, D), np.float32)
    for core in range(8):
        out[core // TP] += res.results[core]["out"].astype(np.float32)
    return out


# revision 60
# speedup vs baseline: 1.0067x; 1.0067x over previous
"""Self-contained Trainium2 Bass kernel for GQA MultiHeadAttention with RoPE.

Problem: B=2, S=2048, D=1024, H=16 Q heads, KVH=4 KV heads, head_dim=64,
causal additive mask, f32.

Sharding: tensor-parallel over heads (TP=4: 4 Q heads + 1 KV head per shard)
x data-parallel over batch (DP=2) = 8 NeuronCores. Wo is sharded on its
input dim; the host sums the 4 partial outputs per batch element.

v3: fp8 hi/lo DoubleRow projections (host splits hT and W into e4m3
hi+lo pairs, W pre-scaled by 32 with the 1/32 folded into the rope
tables and Wo), merged/ordered input DMA plan for an early start,
engine-rebalanced rope/norm/evict chains, split tail DMA.
"""

import os
import sys

for _p in ("/opt/trn_rl_repo", "/root/.axon_site/_ro/trn_rl_repo"):
    if os.path.isdir(_p) and _p not in sys.path:
        sys.path.insert(0, _p)

import numpy as np
import ml_dtypes

import concourse.bacc as bacc
import concourse.bass as bass
import concourse.tile as tile
from concourse import mybir
from concourse.bass_utils import run_bass_kernel_spmd

F32 = mybir.dt.float32
BF16 = mybir.dt.bfloat16
F8 = mybir.dt.float8e4
AF = mybir.ActivationFunctionType
DR = mybir.MatmulPerfMode.DoubleRow

H, KVH, HD = 16, 4, 64
B, S, D = 2, 2048, 1024
TP = 4                      # head-parallel ways
SCALE = HD ** -0.5
NEG = -1e9
NT = S // 128               # 16 kv tiles
NQB = S // 512              # 4 q blocks
WSCL = 32.0                 # weight pre-scale (host), folded back via
                            # rope tables (q,k) and Wo (v)


def _env(k, d):
    return os.environ.get(k, d)


def _patch_act_tables():
    """Make Exp resolve only to natural_log_exp_and_others so the
    act-table-load pass emits one load."""
    from concourse.hw_specs import get_activation_tables
    t = get_activation_tables("gen3")
    for name, fns in t.items():
        if name != "natural_log_exp_and_others":
            fns.discard(AF.Exp)
            fns.discard(AF.Ln)


def _build_nc(causal: bool):
    _patch_act_tables()
    nc = bacc.Bacc()

    # hidden states, hi (c 0:8) and lo (c 8:16) concatenated so one DMA
    # fills both per column slice
    htb = nc.declare_dram_parameter("htb", [128, 16, S], F8, isOutput=False)
    # [p, pp, hl, dc*n] (flattened so DMA slices coalesce)
    wq8 = nc.declare_dram_parameter("wq8", [128, 2, 2, 1024], F8,
                                    isOutput=False)
    # [p, hl, dc*n]
    wkv8 = nc.declare_dram_parameter("wkv8", [128, 2, 1024], F8,
                                     isOutput=False)
    # cos/sin (pre-divided by WSCL), duplicated to 128 partitions: [p, 2, S]
    cs2 = nc.declare_dram_parameter("cs2", [128, 2, S], BF16, isOutput=False)
    wo = nc.declare_dram_parameter("wo", [256, D], BF16, isOutput=False)
    # consts blob: [psig | ident | m01 m01] = [128, 512]
    consts = nc.declare_dram_parameter("consts", [128, 512], BF16,
                                       isOutput=False)
    outp = nc.declare_dram_parameter("out", [S, D], BF16, isOutput=True)

    ev_engines = {
        "dve": lambda: nc.vector,
        "pool": lambda: nc.gpsimd,
    }

    with tile.TileContext(nc) as tc:
        with tc.tile_pool(name="hold", bufs=1) as hp:
            # ---- input DMA plan: first-needed first, two dispatch queues
            # (SP HWDGE for the bulk ht stream, Pool SWDGE for the small
            # tables so dispatches overlap) ----
            wsrc = hp.tile([128, 128], BF16, name="wsrc", tag="wsrc")
            nc.vector.memset(wsrc, 0.0)

            ht_b = hp.tile([128, 16, S], F8, name="ht_b", tag="ht_b")
            ht_hi = ht_b[:, 0:8]
            ht_lo = ht_b[:, 8:16]
            wkv_sb = hp.tile([128, 2, 8, 128], F8, name="wkv_sb",
                             tag="wkv_sb")
            nc.sync.dma_start(
                out=wkv_sb[:, 0].rearrange("p a b -> p (a b)"),
                in_=wkv8[:, 0, :])
            nc.sync.dma_start(out=ht_b[:, 0:8, 0:512],
                              in_=htb[:, 0:8, 0:512])
            nc.sync.dma_start(
                out=wkv_sb[:, 1].rearrange("p a b -> p (a b)"),
                in_=wkv8[:, 1, :])

            con_sb = hp.tile([128, 512], BF16, name="con_sb", tag="con_sb")
            nc.gpsimd.dma_start(out=con_sb, in_=consts[:, :])
            psig_sb = con_sb[:, 0:128]
            id_sb = con_sb[:, 128:256]
            m01_sb = con_sb[:, 256:512].rearrange("p (u c) -> p u c", u=2)

            cs_sb = hp.tile([128, 2, S], BF16, name="cs_sb", tag="cs_sb")
            nc.gpsimd.dma_start(out=cs_sb[:, :, 0:512], in_=cs2[:, :, 0:512])
            cosf_sb = cs_sb[:, 0]
            sinf_sb = cs_sb[:, 1]

            wq_sb = hp.tile([128, 2, 2, 8, 128], F8, name="wq_sb",
                            tag="wq_sb")
            nc.sync.dma_start(
                out=wq_sb[:, 0].rearrange("p a b c -> p (a b c)"),
                in_=wq8[:, 0, :, :].rearrange("p a b -> p (a b)"))
            nc.sync.dma_start(out=ht_b[:, 8:16, 0:512],
                              in_=htb[:, 8:16, 0:512])
            nc.sync.dma_start(
                out=wq_sb[:, 1].rearrange("p a b c -> p (a b c)"),
                in_=wq8[:, 1, :, :].rearrange("p a b -> p (a b)"))

            vsm = hp.tile([128, NT, 65], BF16, name="vsm", tag="vsm")
            nc.vector.memset(vsm[:, :, 64:65], 1.0)

            nc.sync.dma_start(out=ht_b[:, 0:8, 512:1024],
                              in_=htb[:, 0:8, 512:1024])
            nc.sync.dma_start(out=ht_b[:, 8:16, 512:1024],
                              in_=htb[:, 8:16, 512:1024])
            nc.sync.dma_start(out=cs_sb[:, :, 512:1024],
                              in_=cs2[:, :, 512:1024])
            nc.sync.dma_start(out=ht_b[:, 0:8, 1024:2048],
                              in_=htb[:, 0:8, 1024:2048])
            nc.sync.dma_start(out=ht_b[:, 8:16, 1024:2048],
                              in_=htb[:, 8:16, 1024:2048])
            nc.sync.dma_start(out=cs_sb[:, :, 1024:2048],
                              in_=cs2[:, :, 1024:2048])

            wo_sb = hp.tile([128, 2, D], BF16, name="wo_sb", tag="wo_sb")
            nc.sync.dma_start(out=wo_sb,
                              in_=wo.rearrange("(c p) n -> p c n", p=128))

            qTs = [hp.tile([128, S], BF16, name=f"qT{p}", tag=f"qT{p}")
                   for p in range(2)]
            qlo = [hp.tile([64, S], BF16, name=f"qlo{p}", tag=f"qlo{p}")
                   for p in range(2)]
            kT = hp.tile([128, S], BF16, name="kTt", tag="kTt")
            ctxTs = [[hp.tile([128, 512], BF16, name=f"ctxT{c}_{q}",
                              tag=f"ctxT{c}_{q}") for q in range(NQB)]
                     for c in range(2)]

            with tc.tile_pool(name="psS", bufs=1, space="PSUM") as psS, \
                 tc.tile_pool(name="psC", bufs=1, space="PSUM") as psC, \
                 tc.tile_pool(name="psD", bufs=1, space="PSUM") as psD, \
                 tc.tile_pool(name="etp", bufs=1) as etp, \
                 tc.tile_pool(name="sbA", bufs=int(_env("KV2_SAB", "5"))) as sbA, \
                 tc.tile_pool(name="sbC", bufs=1) as sbC:

                # PE warmup burst: dependency-free matmuls ramp the PE
                # clock through the DMA-bound lead
                nwarm = int(_env("V3_WARM", "0"))
                for wi in range(nwarm):
                    ps_w = psS.tile([128, 1024], F32, name="ps_w",
                                    tag="ps_s", bufs=2)[:, 0:128]
                    nc.tensor.matmul(ps_w, wsrc, wsrc,
                                     start=True, stop=True)

                # ---------------- Phase A: projections + rope ----------------
                # staged: proj (matmuls+evict) and rope are emitted at
                # different points so PE never head-of-line blocks on the
                # DVE/Pool rope chain
                raws = {}

                def rope_chunk(dst, raw, npart, csl, late=False):
                    # dst = raw*cos + rot(raw)*sin ; raw: bf16 SBUF [npart,n]
                    n = csl.stop - csl.start
                    if not late and _env("V3_RPS", "0") == "1":
                        # attention psS pool is idle through the head phase
                        ps_rot = psS.tile([128, 1024], F32, name="ps_rot",
                                          tag="ps_s", bufs=2)[0:npart, 0:n]
                    else:
                        ps_rot = psD.tile([128, 512], F32, name="ps_rot",
                                          tag="ps_d", bufs=2)[0:npart, 0:n]
                    nc.tensor.matmul(ps_rot, psig_sb[0:npart, 0:npart],
                                     raw, start=True, stop=True)
                    rmc = _env("V3_RMCL" if late else "V3_RMC", "dve")
                    ev_engines[rmc]().tensor_mul(
                        dst, raw, cosf_sb[0:npart, csl])
                    rtmp = sbA.tile([128, 512], BF16, name="rtmp",
                                    tag="rtmp")[:, 0:n]
                    if _env("V3_ROT", "dve") == "act":
                        rotb = sbA.tile([128, 512], BF16, name="rotb",
                                        tag="rotb")[:, 0:n]
                        nc.scalar.copy(rotb[0:npart, :], ps_rot)
                        ev_engines[_env("V3_RMS", "dve")]().tensor_mul(
                            rtmp[0:npart, :], rotb[0:npart, :],
                            sinf_sb[0:npart, csl])
                    else:
                        rms = _env("V3_RMSL" if late else "V3_RMS", "dve")
                        ev_engines[rms]().tensor_mul(
                            rtmp[0:npart, :], ps_rot, sinf_sb[0:npart, csl])
                    rad = _env("V3_RADL" if late else "V3_RAD", "dve")
                    ev_engines[rad]().tensor_add(
                        dst, dst, rtmp[0:npart, :])

                def qkv_term(ps, w, h, csl, ti):
                    for dc in range(4):
                        nc.tensor.matmul(
                            ps,
                            w[:, 2 * dc:2 * dc + 2, :],
                            h[:, 2 * dc:2 * dc + 2, csl],
                            start=(ti == 0 and dc == 0),
                            stop=(ti == 2 and dc == 3),
                            perf_mode=DR)

                def qkv_mms(ps, w_hi, w_lo, csl):
                    # 12 DoubleRow matmuls: (w_hi,h_hi),(w_lo,h_hi),(w_hi,h_lo)
                    for ti, (w, h) in enumerate(
                            [(w_hi, ht_hi), (w_lo, ht_hi), (w_hi, ht_lo)]):
                        qkv_term(ps, w, h, csl, ti)

                def pevict(dst, ps, late=False):
                    ev = _env("V3_QEVL", "dve") if late else _env("V3_QEV", "act")
                    if ev == "act":
                        nc.scalar.copy(dst, ps)
                    else:
                        nc.vector.tensor_copy(dst, ps)

                def q_proj(pp, sc, half=None):
                    csl = (slice(512 * sc, 512 * sc + 512) if half is None
                           else slice(512 * sc + 256 * half,
                                      512 * sc + 256 * half + 256))
                    n = csl.stop - csl.start
                    ps_q = psD.tile([128, 512], F32, name="ps_q",
                                    tag="ps_d", bufs=2)[:, 0:n]
                    qkv_mms(ps_q, wq_sb[:, pp, 0], wq_sb[:, pp, 1], csl)
                    qraw = sbA.tile([128, 512], BF16, name="qraw",
                                    tag="qraw")[:, 0:n]
                    if sc < 2 and _env("V3_QEVQ", "") == "dve":
                        nc.vector.tensor_copy(qraw, ps_q)
                    else:
                        pevict(qraw, ps_q, late=(sc >= 2))
                    raws[("q", pp, csl.start)] = qraw

                def q_rope(pp, sc, half=None, do_qlo=True):
                    csl = (slice(512 * sc, 512 * sc + 512) if half is None
                           else slice(512 * sc + 256 * half,
                                      512 * sc + 256 * half + 256))
                    qraw = raws.pop(("q", pp, csl.start))
                    rope_chunk(qTs[pp][:, csl], qraw, 128, csl, late=(sc >= 2))
                    # odd head's rows to base 0 so all scores matmuls share
                    # one tile_position row base (mixed bases crash HW)
                    if do_qlo:
                        qsl = csl if half is not None else slice(
                            512 * sc, 512 * sc + 512)
                        nc.sync.dma_start(out=qlo[pp][:, qsl],
                                          in_=qTs[pp][64:128, qsl])

                def kv_proj(sc, half=None):
                    csl = (slice(512 * sc, 512 * sc + 512) if half is None
                           else slice(512 * sc + 256 * half,
                                      512 * sc + 256 * half + 256))
                    n = csl.stop - csl.start
                    ps_kv = psD.tile([128, 512], F32, name="ps_kv",
                                     tag="ps_d", bufs=2)[:, 0:n]
                    qkv_mms(ps_kv, wkv_sb[:, 0], wkv_sb[:, 1], csl)
                    kvraw = sbA.tile([128, 512], BF16, name="kvraw",
                                     tag="kvraw")[:, 0:n]
                    pevict(kvraw, ps_kv, late=(sc >= 2))
                    raws[("kv", sc, csl.start)] = kvraw

                def kv_rope(sc, half=None):
                    csl = (slice(512 * sc, 512 * sc + 512) if half is None
                           else slice(512 * sc + 256 * half,
                                      512 * sc + 256 * half + 256))
                    kvraw = raws.pop(("kv", sc, csl.start))
                    # rope on K rows 0:64
                    rope_chunk(kT[0:64, csl], kvraw[0:64, :], 64, csl, late=(sc >= 2))
                    # V rows 64:128: transpose each 128-seq tile into vsm
                    ntt = (csl.stop - csl.start) // 128
                    use_dma = _env("V3_VT", "dma") == "dma" and sc >= 1
                    for tt in range(ntt):
                        ti = csl.start // 128 + tt
                        if use_dma:
                            # XBAR transpose needs a contiguous destination
                            # (strided dst produces wrong output on HW), so
                            # bounce through a temp tile and strided-copy
                            # into vsm via Pool SWDGE
                            vtmp = sbA.tile([128, 64], BF16, name="vtmp",
                                            tag="vtmp", bufs=4)
                            nc.sync.dma_start_transpose(
                                vtmp,
                                kvraw[64:128, 128 * tt:128 * tt + 128])
                            nc.sync.dma_start(out=vsm[:, ti, 0:64],
                                              in_=vtmp)
                            continue
                        ps_v = psD.tile([128, 512], BF16, name="ps_v",
                                        tag="ps_d", bufs=2)[:, 0:64]
                        nc.tensor.matmul(
                            ps_v,
                            kvraw[64:128, 128 * tt:128 * tt + 128],
                            id_sb[64:128, 0:64],
                            start=True, stop=True, is_transpose=True)
                        ev_engines[_env("V3_VEV", "dve")]().tensor_copy(
                            vsm[:, ti, 0:64], ps_v)

                def q_proj_stages(pp, sc):
                    csl = slice(512 * sc, 512 * sc + 512)
                    box = {}

                    def c1():
                        box["ps"] = psD.tile([128, 512], F32, name="ps_q",
                                             tag="ps_d", bufs=2)
                        qkv_term(box["ps"], wq_sb[:, pp, 0], ht_hi, csl, 0)

                    def c2():
                        qkv_term(box["ps"], wq_sb[:, pp, 1], ht_hi, csl, 1)

                    def c3():
                        qkv_term(box["ps"], wq_sb[:, pp, 0], ht_lo, csl, 2)
                        qraw = sbA.tile([128, 512], BF16, name="qraw",
                                        tag="qraw")
                        if sc < 2 and _env("V3_QEVQ", "") == "dve":
                            nc.vector.tensor_copy(qraw, box["ps"])
                        else:
                            pevict(qraw, box["ps"], late=(sc >= 2))
                        raws[("q", pp, csl.start)] = qraw
                    return [c1, c2, c3]

                def kv_proj_stages(sc):
                    csl = slice(512 * sc, 512 * sc + 512)
                    box = {}

                    def c1():
                        box["ps"] = psD.tile([128, 512], F32, name="ps_kv",
                                             tag="ps_d", bufs=2)
                        qkv_term(box["ps"], wkv_sb[:, 0], ht_hi, csl, 0)

                    def c2():
                        qkv_term(box["ps"], wkv_sb[:, 1], ht_hi, csl, 1)

                    def c3():
                        qkv_term(box["ps"], wkv_sb[:, 0], ht_lo, csl, 2)
                        kvraw = sbA.tile([128, 512], BF16, name="kvraw",
                                         tag="kvraw")
                        pevict(kvraw, box["ps"], late=(sc >= 2))
                        raws[("kv", sc, csl.start)] = kvraw
                    return [c1, c2, c3]

                def emit_q_sc(pp, sc):
                    q_proj(pp, sc)
                    q_rope(pp, sc)

                def emit_kv_sc(sc):
                    kv_proj(sc)
                    kv_rope(sc)

                def ost_evict(ost, nb, ps_o, tail=False):
                    # tail D-evicts go to ACT (exp is done by then)
                    if tail and nb == 1:
                        if _env("V3_TEV", "act") == "act":
                            nc.scalar.copy(ost[:, 512 * nb:512 * nb + 512],
                                           ps_o)
                        else:
                            nc.vector.tensor_copy(
                                ost[:, 512 * nb:512 * nb + 512], ps_o)
                        return
                    oev = _env("V3_OEV", "dve")
                    if oev == "act":
                        nc.scalar.copy(ost[:, 512 * nb:512 * nb + 512], ps_o)
                    else:
                        ev_engines[oev]().tensor_copy(
                            ost[:, 512 * nb:512 * nb + 512], ps_o)

                def emit_d_qt(qb, qt):
                    ost = sbC.tile([128, 1024], BF16, name="ost", tag="ost",
                                   bufs=int(_env("KV2_OSTB", "8")))
                    for nb in range(2):
                        ps_o = psD.tile([128, 512], F32, name="ps_o",
                                        tag="ps_d", bufs=2)
                        for c in range(2):
                            nc.tensor.matmul(
                                ps_o,
                                ctxTs[c][qb][:, 128 * qt:128 * qt + 128],
                                wo_sb[:, c, 512 * nb:512 * nb + 512],
                                start=(c == 0), stop=(c == 1))
                        ost_evict(ost, nb, ps_o)
                    row = 512 * qb + 128 * qt
                    nc.sync.dma_start(out=outp[row:row + 128, :], in_=ost)

                tail_ps_n = [0]

                def emit_phase_d(qb, qts, tail=False):
                    for qt in qts:
                        ost = sbC.tile([128, 1024], BF16, name="ost",
                                       tag="ost",
                                       bufs=int(_env("KV2_OSTB", "8")))
                        split = (tail and (
                            (qb == 3 and qt == qts[-1]) or
                            _env("V3_TAS", "0") == "1") and
                            _env("V3_LS", "0") == "1")
                        for nb in range(2):
                            if tail and _env("V3_TPS", "1") == "1":
                                # attention psum pools are free by the tail:
                                # rotate D psums through them so evicts
                                # never gate the next matmul. psC is held
                                # by the final norm chains - delay its use.
                                i = tail_ps_n[0]
                                tail_ps_n[0] += 1
                                tv = _env("V3_TPSV", "a")
                                if tv == "nc":
                                    r = [0, 2][i % 2]
                                elif tv == "c2":
                                    r = [0, 2, 0, 2, 1][i % 5] if i >= 2 \
                                        else [0, 2][i]
                                else:
                                    r = [0, 2, 0, 2][i] if i < 4 else (i % 3)
                                if r == 0:
                                    ps_o = psS.tile([128, 1024], F32,
                                                    name="ps_o", tag="ps_s",
                                                    bufs=2)[:, 0:512]
                                elif r == 1:
                                    ps_o = psC.tile([128, 512], F32,
                                                    name="ps_o",
                                                    tag="ps_ctx", bufs=2)
                                else:
                                    ps_o = psD.tile([128, 512], F32,
                                                    name="ps_o", tag="ps_d",
                                                    bufs=2)
                            else:
                                ps_o = psD.tile([128, 512], F32, name="ps_o",
                                                tag="ps_d", bufs=2)
                            for c in range(2):
                                ct = ctxTs[c][qb]
                                col = 128 * qt
                                nc.tensor.matmul(
                                    ps_o,
                                    ct[:, col:col + 128],
                                    wo_sb[:, c, 512 * nb:512 * nb + 512],
                                    start=(c == 0), stop=(c == 1))
                            ost_evict(ost, nb, ps_o, tail=tail)
                            if split:
                                # pipeline the last tile's DMA per-half so
                                # the final transfer is short; first half
                                # goes out via Pool SWDGE so the final
                                # HWDGE dispatch isn't queued behind it
                                row = 512 * qb + 128 * qt
                                last = qb == 3 and qt == qts[-1]
                                eng = (nc.gpsimd if nb == 0 and last and
                                       _env("V3_TSP", "1") == "1"
                                       else nc.sync)
                                eng.dma_start(
                                    out=outp[row:row + 128,
                                             512 * nb:512 * nb + 512],
                                    in_=ost[:, 512 * nb:512 * nb + 512])
                        if not split:
                            row = 512 * qb + 128 * qt
                            eng = (nc.gpsimd if tail and qb == 3 and
                                   qt == 2 and _env("V3_T2P", "0") == "1"
                                   else nc.sync)
                            eng.dma_start(
                                out=outp[row:row + 128, :],
                                in_=ost)

                def norm_tail(h, qb, ps_ctx):
                    # normalization; split mode evicts the unnormalized ctx
                    # immediately (ACT) so the psC slot frees in ~0.7us
                    # instead of holding through the recip/bcast/mul chain
                    rs = sbC.tile([1, 512], F32, name="rs", tag="rs",
                                  bufs=int(_env("KV2_RSB", "3")))
                    nc.vector.reciprocal(rs, ps_ctx[64:65, :])
                    rb = sbC.tile([64, 512], F32, name="rb", tag="rb",
                                  bufs=int(_env("KV2_RBB", "6")))
                    nc.gpsimd.partition_broadcast(rb, rs, channels=64)
                    c = h % 2
                    nrm = ev_engines[_env("V3_NRM", "dve")]()
                    split = _env("V3_NSPLIT", "0") == "1"
                    if split:
                        cu = sbC.tile([64, 512], BF16, name="cu", tag="cu",
                                      bufs=int(_env("V3_CUB", "3")))
                        if _env("V3_CUE", "act") == "act":
                            nc.scalar.copy(cu, ps_ctx[0:64, :])
                        else:
                            nc.vector.tensor_copy(cu, ps_ctx[0:64, :])
                        csrc = cu
                    else:
                        csrc = ps_ctx[0:64, :]
                    if h < 2:
                        nrm.tensor_mul(ctxTs[c][qb][0:64, :], csrc, rb)
                    else:
                        csh = sbC.tile([64, 512], BF16, name="csh",
                                       tag="csh",
                                       bufs=int(_env("KV2_CSB", "2")))
                        nrm.tensor_mul(csh, csrc, rb)
                        nc.sync.dma_start(out=ctxTs[c][qb][64:128, :],
                                          in_=csh)

                def emit_bc2(qb, ha, hb, mid=(), fillers=()):
                    # two heads sharing one diag-B psum+exp unit: head X's
                    # diag tiles j2,j3 at et cols [0:384], head Y's at
                    # [384:768] -> one [128,768] exp instead of two [128,384]
                    qsl = slice(512 * qb, 512 * qb + 512)
                    nfull = 4 * qb
                    nki = 4 * qb + 4
                    shared = {}

                    # region layout keeps every matmul output inside one
                    # psum bank: ha-j2 [0:256], ha-j3 [256:384],
                    # hb-j3 [384:512], hb-j2 [512:768]
                    REG = {(0, 2): 0, (0, 3): 256, (1, 3): 384, (1, 2): 512}

                    def s_B():
                        ps_sB = psS.tile([128, 1024], F32, name="ps_s",
                                         tag="ps_s", bufs=2)
                        for idx, h in ((0, ha), (1, hb)):
                            pp = h // 2
                            qsrc = qTs[pp] if h % 2 == 0 else qlo[pp]
                            for j in (2, 3):
                                cl = REG[(idx, j)]
                                ki = 4 * qb + j
                                span = 512 - 128 * j
                                nc.tensor.matmul(
                                    ps_sB[:, cl:cl + span],
                                    kT[0:64, 128 * ki:128 * ki + 128],
                                    qsrc[0:64,
                                         512 * qb + 128 * j:512 * (qb + 1)],
                                    start=True, stop=True)
                        shared["ps"] = ps_sB

                    def e_B():
                        etB = etp.tile([128, 1024], BF16, name="et", tag="et",
                                       bufs=int(_env("KV2_ETB", "34")))
                        nc.scalar.activation(etB[:, 0:768],
                                             shared["ps"][:, 0:768],
                                             AF.Exp, scale=SCALE)
                        # triangles: ha at 0 & 256 (stride 256), hb at 384 &
                        # 512 (stride 128)
                        tri_eng = ev_engines[_env("V3_TRI", "dve")]()
                        tri_a = etB[:, 0:512].rearrange(
                            "p (u c) -> p u c", u=2)[:, :, 0:128]
                        tri_eng.tensor_mul(tri_a, tri_a, m01_sb)
                        tri_b = etB[:, 384:640].rearrange(
                            "p (u c) -> p u c", u=2)
                        tri_eng.tensor_mul(tri_b, tri_b, m01_sb)
                        shared["et"] = etB

                    def ctx_B(idx, ps_ctx, vsl):
                        for j in (2, 3):
                            cl = REG[(idx, j)]
                            ki = 4 * qb + j
                            span = 512 - 128 * j
                            nc.tensor.matmul(
                                ps_ctx[:, 128 * j:512],
                                vsm[:, ki, vsl],
                                shared["et"][:, cl:cl + span],
                                start=(ki == 0), stop=(ki == nki - 1))

                    def head_part(idx, h):
                        pp = h // 2
                        qsrc = qTs[pp] if h % 2 == 0 else qlo[pp]
                        ps_ctx = psC.tile([128, 512], F32, name="ps_ctx",
                                          tag="ps_ctx", bufs=2)[0:65, :]
                        vsl = slice(0, 65)
                        units = []

                        def mk_pair(kp):
                            box = {}

                            def s():
                                ps_s = psS.tile([128, 1024], F32, name="ps_s",
                                                tag="ps_s", bufs=2)
                                for jj in range(2):
                                    ki = kp + jj
                                    nc.tensor.matmul(
                                        ps_s[:, 512 * jj:512 * jj + 512],
                                        kT[0:64, 128 * ki:128 * ki + 128],
                                        qsrc[0:64, qsl],
                                        start=True, stop=True)
                                box["ps"] = ps_s

                            def ec():
                                et = etp.tile([128, 1024], BF16, name="et",
                                              tag="et",
                                              bufs=int(_env("KV2_ETB", "34")))
                                nc.scalar.activation(et, box["ps"], AF.Exp,
                                                     scale=SCALE)
                                for jj in range(2):
                                    ki = kp + jj
                                    nc.tensor.matmul(
                                        ps_ctx,
                                        vsm[:, ki, vsl],
                                        et[:, 512 * jj:512 * jj + 512],
                                        start=(ki == 0), stop=(ki == nki - 1))
                            return (s, ec)

                        def mk_diag_a():
                            box = {}

                            def s():
                                ps_s = psS.tile([128, 1024], F32, name="ps_s",
                                                tag="ps_s", bufs=2)
                                for j, cl in ((0, 0), (1, 512)):
                                    ki = 4 * qb + j
                                    span = 512 - 128 * j
                                    nc.tensor.matmul(
                                        ps_s[:, cl:cl + span],
                                        kT[0:64, 128 * ki:128 * ki + 128],
                                        qsrc[0:64,
                                             512 * qb + 128 * j:512 * (qb + 1)],
                                        start=True, stop=True)
                                box["ps"] = ps_s

                            def ec():
                                et = etp.tile([128, 1024], BF16, name="et",
                                              tag="et",
                                              bufs=int(_env("KV2_ETB", "34")))
                                nc.scalar.activation(et[:, 0:896],
                                                     box["ps"][:, 0:896],
                                                     AF.Exp, scale=SCALE)
                                tri = et[:, 0:1024].rearrange(
                                    "p (u c) -> p u c", u=2)[:, :, 0:128]
                                ev_engines[_env("V3_TRI", "dve")]().tensor_mul(
                                    tri, tri, m01_sb)
                                for j, cl in ((0, 0), (1, 512)):
                                    ki = 4 * qb + j
                                    span = 512 - 128 * j
                                    nc.tensor.matmul(
                                        ps_ctx[:, 128 * j:512],
                                        vsm[:, ki, vsl],
                                        et[:, cl:cl + span],
                                        start=(ki == 0), stop=(ki == nki - 1))
                            return (s, ec)

                        for kp in range(0, nfull, 2):
                            units.append(mk_pair(kp))
                        units.append(mk_diag_a())
                        if idx == 0:
                            units.append((s_B, e_B))
                        else:
                            units.append((lambda: None,
                                          lambda: ctx_B(1, ps_ctx, vsl)))
                        units[0][0]()
                        fcad = int(_env("V3_FCAD", "2"))
                        for i in range(len(units)):
                            if i + 1 < len(units):
                                units[i + 1][0]()
                            units[i][1]()
                            if fil and i % fcad == fcad - 1:
                                fil.pop(0)()
                        if idx == 0:
                            ctx_B(0, ps_ctx, vsl)

                        norm_tail(h, qb, ps_ctx)

                    fil = list(fillers)
                    head_part(0, ha)
                    for m in mid:
                        m()
                    head_part(1, hb)
                    for f in fil:
                        f()

                def emit_bc(qb, h, fillers=()):
                    # attention for one (q block, head): scores, exp, ctx,
                    # normalization fused into the evict
                    pp = h // 2
                    qsrc = qTs[pp] if h % 2 == 0 else qlo[pp]
                    qsl = slice(512 * qb, 512 * qb + 512)
                    ps_ctx = psC.tile([128, 512], F32, name="ps_ctx",
                                      tag="ps_ctx", bufs=2)[0:65, :]
                    vsl = slice(0, 65)
                    nfull = (4 * qb) if causal else NT
                    nki = (4 * qb + 4) if causal else NT
                    units = []

                    def mk_pair(kp):
                        # full pair unit: kv tiles (kp, kp+1)
                        box = {}

                        def s():
                            ps_s = psS.tile([128, 1024], F32, name="ps_s",
                                            tag="ps_s", bufs=2)
                            for jj in range(2):
                                ki = kp + jj
                                nc.tensor.matmul(
                                    ps_s[:, 512 * jj:512 * jj + 512],
                                    kT[0:64, 128 * ki:128 * ki + 128],
                                    qsrc[0:64, qsl],
                                    start=True, stop=True)
                            box["ps"] = ps_s

                        def ec():
                            et = etp.tile([128, 1024], BF16, name="et",
                                          tag="et",
                                          bufs=int(_env("KV2_ETB", "34")))
                            nc.scalar.activation(et, box["ps"], AF.Exp,
                                                 scale=SCALE)
                            for jj in range(2):
                                ki = kp + jj
                                nc.tensor.matmul(
                                    ps_ctx,
                                    vsm[:, ki, vsl],
                                    et[:, 512 * jj:512 * jj + 512],
                                    start=(ki == 0), stop=(ki == nki - 1))
                        return (s, ec)

                    def mk_diag(du):
                        # diag unit du=0: tiles j=0,1 at cols 0 / 512
                        #           du=1: tiles j=2,3 at cols 0 / 256
                        box = {}
                        js = (0, 1) if du == 0 else (2, 3)
                        cols = (0, 512) if du == 0 else (0, 256)
                        wid = 896 if du == 0 else 384

                        def s():
                            ps_s = psS.tile([128, 1024], F32, name="ps_s",
                                            tag="ps_s", bufs=2)
                            for j, cl in zip(js, cols):
                                ki = 4 * qb + j
                                span = 512 - 128 * j
                                nc.tensor.matmul(
                                    ps_s[:, cl:cl + span],
                                    kT[0:64, 128 * ki:128 * ki + 128],
                                    qsrc[0:64,
                                         512 * qb + 128 * j:512 * (qb + 1)],
                                    start=True, stop=True)
                            box["ps"] = ps_s

                        def ec():
                            et = etp.tile([128, 1024], BF16, name="et",
                                          tag="et",
                                          bufs=int(_env("KV2_ETB", "34")))
                            nc.scalar.activation(et[:, 0:wid],
                                                 box["ps"][:, 0:wid],
                                                 AF.Exp, scale=SCALE)
                            # zero the causal triangles: both triangles in
                            # this unit are 128 wide, cols[1] apart
                            tri = et[:, 0:2 * cols[1]].rearrange(
                                "p (u c) -> p u c", u=2)[:, :, 0:128]
                            nc.vector.tensor_mul(tri, tri, m01_sb)
                            for j, cl in zip(js, cols):
                                ki = 4 * qb + j
                                span = 512 - 128 * j
                                nc.tensor.matmul(
                                    ps_ctx[:, 128 * j:512],
                                    vsm[:, ki, vsl],
                                    et[:, cl:cl + span],
                                    start=(ki == 0), stop=(ki == nki - 1))
                        return (s, ec)

                    for kp in range(0, nfull, 2):
                        units.append(mk_pair(kp))
                    if causal:
                        units.append(mk_diag(0))
                        units.append(mk_diag(1))
                    if units:
                        units[0][0]()
                    fi = list(fillers)
                    for i in range(len(units)):
                        if i + 1 < len(units):
                            units[i + 1][0]()
                        units[i][1]()
                        # interleave ready phase-D tiles into the unit
                        # sequence so they fill PE gaps while ACT paces
                        if fi and i % 2 == 1:
                            fi.pop(0)()
                    for f in fi:
                        f()

                    norm_tail(h, qb, ps_ctx)

                # ---- global emission order ----
                # attention on q-block qb needs rope chunks 0..qb of q/k;
                # A-phase chunks are threaded through the bc groups so the
                # rope for chunk sc overlaps attention on earlier blocks
                gv = _env("KV2_GVAR", "7")
                ho = [int(x) for x in _env("KV2_HORD", "0123")]
                if gv == "7":
                    # pipelined head, staged proj/rope; first bc2 of each
                    # block pairs the EVEN heads (0,2) so no qlo DMA is on
                    # the critical path
                    if _env("V3_HALF", "0") == "1":
                        kv_proj(0, half=0)
                        kv_proj(0, half=1)
                        q_proj(0, 0, half=0)
                        kv_rope(0, half=0)
                        q_proj(0, 0, half=1)
                        kv_rope(0, half=1)
                        q_rope(0, 0, half=0)
                        q_proj(1, 0, half=0)
                        q_rope(0, 0, half=1)
                        q_proj(1, 0, half=1)
                        q_rope(1, 0, half=0)
                        q_rope(1, 0, half=1)
                    else:
                        hv = _env("V3_HV", "5")
                        if hv == "7":
                            # like 6 but kT's rope follows the kv evict
                            # immediately so the K chain isn't delayed
                            kvs = kv_proj_stages(0)
                            qs = q_proj_stages(0, 0)
                            kvs[0]()
                            kvs[1]()
                            qs[0]()
                            qs[1]()
                            kvs[2]()
                            kv_rope(0)
                            qs[2]()
                            q_rope(0, 0)
                        elif hv == "6":
                            # interleave kv/q projection terms so q's
                            # hi-terms run while kv's lo-terms wait on htl
                            kvs = kv_proj_stages(0)
                            qs = q_proj_stages(0, 0)
                            kvs[0]()
                            kvs[1]()
                            qs[0]()
                            qs[1]()
                            kvs[2]()
                            qs[2]()
                            kv_rope(0)
                            q_rope(0, 0)
                        else:
                            kv_proj(0)
                            q_proj(0, 0)
                            kv_rope(0)
                            if hv not in ("5", "6"):
                                q_proj(1, 0)
                                q_rope(0, 0)
                                q_rope(1, 0)
                            else:
                                q_rope(0, 0)
                    hpair = _env("V3_PAIR", "2031")
                    if hpair == "02":
                        pairs = ((0, 2), (1, 3))
                    elif hpair == "20":
                        pairs = ((2, 0), (1, 3))
                    elif hpair == "0231":
                        pairs = ((0, 2), (3, 1))
                    elif hpair == "2031":
                        pairs = ((2, 0), (3, 1))
                    else:
                        pairs = ((0, 1), (2, 3))
                    use_fil = _env("V3_AFIL", "0") == "1"
                    dv15 = _env("KV2_DVAR", "13") == "15"
                    for qb in range(3):
                        fil2 = ([(lambda qt=qt: emit_d_qt(0, qt))
                                 for qt in range(4)]
                                if (dv15 and qb == 2) else [])
                        if qb == 0 and _env("V3_HV", "5") in ("5", "6", "7"):
                            # head 2 only needs qT1 at the second
                            # head_part: fold q1's proj+rope into mid so
                            # attention starts as soon as qT0 is roped
                            if _env("V3_HV5M", "a") == "b":
                                emit_bc2(0, *pairs[0],
                                         mid=[lambda: kv_proj(1),
                                              lambda: q_proj(1, 0),
                                              lambda: q_rope(1, 0),
                                              lambda: q_proj(0, 1)])
                            else:
                                emit_bc2(0, *pairs[0],
                                         mid=[lambda: q_proj(1, 0),
                                              lambda: kv_proj(1),
                                              lambda: q_rope(1, 0),
                                              lambda: q_proj(0, 1)])
                            emit_bc2(0, *pairs[1],
                                     mid=[lambda: kv_rope(1),
                                          lambda: q_rope(0, 1),
                                          lambda: q_proj(1, 1)],
                                     fillers=fil2)
                            q_rope(1, 1)
                            continue
                        if use_fil:
                            emit_bc2(qb, *pairs[0],
                                     fillers=(kv_proj_stages(qb + 1)
                                              + q_proj_stages(0, qb + 1)))
                            emit_bc2(qb, *pairs[1],
                                     mid=[lambda qb=qb: kv_rope(qb + 1),
                                          lambda qb=qb: q_rope(0, qb + 1)],
                                     fillers=q_proj_stages(1, qb + 1) + fil2)
                        else:
                            mv = _env("V3_MIDV", "A")
                            if mv == "E":
                                emit_bc2(qb, *pairs[0],
                                         mid=[lambda qb=qb: kv_proj(qb + 1),
                                              lambda qb=qb: q_proj(0, qb + 1),
                                              lambda qb=qb: kv_rope(qb + 1)])
                                emit_bc2(qb, *pairs[1],
                                         mid=[lambda qb=qb: q_rope(0, qb + 1),
                                              lambda qb=qb: q_proj(1, qb + 1)],
                                         fillers=fil2)
                            elif mv == "F":
                                emit_bc2(qb, *pairs[0],
                                         mid=[lambda qb=qb: kv_proj(qb + 1),
                                              lambda qb=qb: q_proj(0, qb + 1)])
                                emit_bc2(qb, *pairs[1],
                                         mid=[lambda qb=qb: q_proj(1, qb + 1),
                                              lambda qb=qb: kv_rope(qb + 1),
                                              lambda qb=qb: q_rope(0, qb + 1)],
                                         fillers=fil2)
                            elif mv == "B":
                                emit_bc2(qb, *pairs[0],
                                         mid=[lambda qb=qb: kv_proj(qb + 1),
                                              lambda qb=qb: q_proj(0, qb + 1),
                                              lambda qb=qb: q_proj(1, qb + 1)])
                                emit_bc2(qb, *pairs[1],
                                         mid=[lambda qb=qb: kv_rope(qb + 1),
                                              lambda qb=qb: q_rope(0, qb + 1)],
                                         fillers=fil2)
                            elif mv == "D":
                                emit_bc2(qb, *pairs[0],
                                         mid=[lambda qb=qb: kv_proj(qb + 1),
                                              lambda qb=qb: q_proj(0, qb + 1),
                                              lambda qb=qb: q_proj(1, qb + 1)])
                                emit_bc2(qb, *pairs[1],
                                         mid=[lambda qb=qb: kv_rope(qb + 1),
                                              lambda qb=qb: q_rope(0, qb + 1),
                                              lambda qb=qb: q_rope(1, qb + 1)],
                                         fillers=fil2)
                            else:
                                emit_bc2(qb, *pairs[0],
                                         mid=[lambda qb=qb: kv_proj(qb + 1),
                                              lambda qb=qb: q_proj(0, qb + 1)])
                                emit_bc2(qb, *pairs[1],
                                         mid=[lambda qb=qb: kv_rope(qb + 1),
                                              lambda qb=qb: q_rope(0, qb + 1),
                                              lambda qb=qb: q_proj(1, qb + 1)],
                                         fillers=fil2)
                        if _env("V3_MIDV", "A") != "D":
                            q_rope(1, qb + 1)
                else:
                    emit_kv_sc(0)
                    emit_q_sc(0, 0)
                    emit_q_sc(1, 0)
                if gv == "7":
                    pass
                elif gv == "6":
                    for qb in range(3):
                        emit_bc2(qb, 0, 1,
                                 mid=[lambda qb=qb: emit_kv_sc(qb + 1)])
                        emit_bc2(qb, 2, 3,
                                 mid=[lambda qb=qb: emit_q_sc(0, qb + 1)])
                        emit_q_sc(1, qb + 1)
                elif gv == "5":
                    for qb in range(3):
                        emit_bc2(qb, 0, 1)
                        emit_kv_sc(qb + 1)
                        emit_bc2(qb, 2, 3,
                                 mid=[lambda qb=qb: emit_q_sc(0, qb + 1)])
                        emit_q_sc(1, qb + 1)
                elif gv == "8":
                    for qb in range(3):
                        emit_bc2(qb, 0, 1,
                                 mid=[lambda qb=qb: emit_kv_sc(qb + 1)])
                        emit_bc2(qb, 2, 3,
                                 mid=[lambda qb=qb: emit_q_sc(0, qb + 1),
                                      lambda qb=qb: emit_q_sc(1, qb + 1)])
                else:
                    for qb in range(3):
                        emit_bc(qb, ho[0])
                        emit_bc(qb, ho[1])
                        emit_bc(qb, ho[2])
                        emit_kv_sc(qb + 1)
                        emit_bc(qb, ho[3])
                        emit_q_sc(0, qb + 1)
                        emit_q_sc(1, qb + 1)
                dv = int(_env("KV2_DVAR", "13"))
                tp = _env("V3_TPAIR", "23_01")
                tpairs = {"13_20": ((1, 3), (2, 0)), "23_10": ((2, 3), (1, 0)),
                          "23_01": ((2, 3), (0, 1)), "31_20": ((3, 1), (2, 0)),
                          "32_01": ((3, 2), (0, 1)),
                          "23_01b": ((2, 3), (0, 1))}[tp]
                if dv == 11:
                    emit_bc2(3, *tpairs[0])
                    emit_phase_d(0, [0, 1, 2, 3])
                    emit_bc2(3, *tpairs[1])
                    emit_phase_d(1, [0, 1, 2, 3])
                    emit_phase_d(2, [0, 1, 2, 3], tail=True)
                    emit_phase_d(3, [0, 1, 2, 3], tail=True)
                elif dv == 10:
                    emit_bc2(3, 1, 2)
                    emit_phase_d(0, [0, 1, 2, 3])
                    emit_bc2(3, 3, 0)
                    emit_phase_d(1, [0, 1, 2, 3])
                    emit_phase_d(2, [0, 1, 2, 3], tail=True)
                    emit_phase_d(3, [0, 1, 2, 3], tail=True)
                elif dv == 15:
                    emit_bc2(3, *tpairs[0],
                             fillers=[(lambda qt=qt: emit_d_qt(1, qt))
                                      for qt in range(4)])
                    emit_bc2(3, *tpairs[1],
                             fillers=[(lambda qt=qt: emit_d_qt(2, qt))
                                      for qt in range(4)])
                    emit_phase_d(3, [0, 1, 2, 3], tail=True)
                elif dv == 13:
                    emit_bc2(3, *tpairs[0],
                             fillers=[(lambda qt=qt: emit_d_qt(0, qt))
                                      for qt in range(4)])
                    emit_bc2(3, *tpairs[1],
                             fillers=[(lambda qt=qt: emit_d_qt(1, qt))
                                      for qt in range(4)]
                             + [(lambda qt=qt: emit_d_qt(2, qt))
                                for qt in range(2)])
                    emit_phase_d(2, [2, 3], tail=True)
                    emit_phase_d(3, [0, 1, 2, 3], tail=True)
                elif dv == 19:
                    emit_bc2(3, *tpairs[0],
                             fillers=[(lambda qt=qt: emit_d_qt(0, qt))
                                      for qt in range(4)]
                             + [(lambda qt=qt: emit_d_qt(1, qt))
                                for qt in range(2)])
                    emit_bc2(3, *tpairs[1],
                             fillers=[(lambda qt=qt: emit_d_qt(1, qt))
                                      for qt in range(2, 4)]
                             + [(lambda qt=qt: emit_d_qt(2, qt))
                                for qt in range(4)])
                    emit_phase_d(3, [0, 1, 2, 3], tail=True)
                elif dv == 18:
                    emit_bc2(3, *tpairs[0],
                             fillers=[(lambda qt=qt: emit_d_qt(0, qt))
                                      for qt in range(3)])
                    emit_d_qt(0, 3)
                    emit_bc2(3, *tpairs[1],
                             fillers=[(lambda qt=qt: emit_d_qt(1, qt))
                                      for qt in range(4)]
                             + [(lambda qt=qt: emit_d_qt(2, qt))
                                for qt in range(2)])
                    emit_phase_d(2, [2, 3], tail=True)
                    emit_phase_d(3, [0, 1, 2, 3], tail=True)
                elif dv == 17:
                    emit_bc2(3, *tpairs[0],
                             fillers=[(lambda qt=qt: emit_d_qt(0, qt))
                                      for qt in range(4)])
                    emit_bc2(3, *tpairs[1],
                             fillers=[(lambda qt=qt: emit_d_qt(1, qt))
                                      for qt in range(4)]
                             + [lambda: emit_d_qt(2, 0)])
                    emit_phase_d(2, [1, 2, 3], tail=True)
                    emit_phase_d(3, [0, 1, 2, 3], tail=True)
                elif dv == 16:
                    emit_bc2(3, *tpairs[0],
                             fillers=[(lambda qt=qt: emit_d_qt(0, qt))
                                      for qt in range(4)])
                    emit_bc2(3, *tpairs[1],
                             fillers=[(lambda qt=qt: emit_d_qt(1, qt))
                                      for qt in range(4)])
                    emit_phase_d(2, [0, 1, 2, 3], tail=True)
                    emit_phase_d(3, [0, 1, 2, 3], tail=True)
                elif dv == 14:
                    emit_bc2(3, *tpairs[0],
                             fillers=[(lambda qt=qt: emit_d_qt(0, qt))
                                      for qt in range(4)])
                    emit_bc2(3, *tpairs[1],
                             fillers=[(lambda qt=qt: emit_d_qt(1, qt))
                                      for qt in range(4)]
                             + [(lambda qt=qt: emit_d_qt(2, qt))
                                for qt in range(4)])
                    emit_phase_d(3, [0, 1, 2, 3], tail=True)
                elif dv == 12:
                    emit_bc2(3, 1, 2,
                             mid=[lambda: emit_phase_d(0, [0, 1])])
                    emit_phase_d(0, [2, 3])
                    emit_bc2(3, 3, 0,
                             mid=[lambda: emit_phase_d(1, [0, 1])])
                    emit_phase_d(1, [2, 3])
                    emit_phase_d(2, [0, 1, 2, 3], tail=True)
                    emit_phase_d(3, [0, 1, 2, 3], tail=True)
                else:
                    emit_bc(3, 0)
                    emit_bc(3, 1)
                    emit_phase_d(0, [0, 1, 2, 3])
                    emit_bc(3, 2)
                    emit_phase_d(1, [0, 1, 2, 3])
                    emit_bc(3, 3)
                    emit_phase_d(2, [0, 1, 2, 3])
                    emit_phase_d(3, [0, 1, 2, 3], tail=True)

    nc.compile()
    return nc


_NC_CACHE = {}


def _get_nc(causal: bool):
    if causal not in _NC_CACHE:
        _NC_CACHE[causal] = _build_nc(causal)
    return _NC_CACHE[causal]


def _host_consts():
    p = np.zeros((128, 128), np.float32)
    idx = np.arange(0, 128, 2)
    p[idx, idx + 1] = -1.0
    p[idx + 1, idx] = 1.0
    psigT = np.ascontiguousarray(p.T)
    ident = np.zeros((128, 128), np.float32)
    ident[64:128, 0:64] = np.eye(64, dtype=np.float32)
    m01 = (np.arange(128)[None, :] >= np.arange(128)[:, None]).astype(np.float32)
    return np.concatenate([psigT, ident, m01, m01], axis=1)


def _numpy_reference(hidden_states, cos, sin, attention_mask, Wq, Wk, Wv, Wo):
    """Generic-mask fallback, pure numpy port of the reference."""
    GROUPS = H // KVH

    def rope(x, c, s):
        c = c[:, None, :, :]
        s = s[:, None, :, :]
        x1, x2 = x[..., ::2], x[..., 1::2]
        xr = np.stack([x1 * c - x2 * s, x1 * s + x2 * c], axis=-1)
        return xr.reshape(x.shape)

    b, sq, d = hidden_states.shape
    q = (hidden_states @ Wq).reshape(b, sq, H, HD).transpose(0, 2, 1, 3)
    k = (hidden_states @ Wk).reshape(b, sq, KVH, HD).transpose(0, 2, 1, 3)
    v = (hidden_states @ Wv).reshape(b, sq, KVH, HD).transpose(0, 2, 1, 3)
    q = rope(q, cos, sin)
    k = rope(k, cos, sin)
    k = np.repeat(k, GROUPS, axis=1)
    v = np.repeat(v, GROUPS, axis=1)
    out = np.zeros((b, sq, d), np.float32)
    for bi in range(b):
        for hi in range(H):
            sc = (q[bi, hi] @ k[bi, hi].T) * SCALE + attention_mask[0, 0]
            sc = sc - sc.max(axis=-1, keepdims=True)
            e = np.exp(sc)
            pr = e / e.sum(axis=-1, keepdims=True)
            ctx = pr @ v[bi, hi]
            out[bi] += ctx @ Wo[hi * HD:(hi + 1) * HD]
    return out


def _split8(x):
    f8 = ml_dtypes.float8_e4m3
    hi = x.astype(f8)
    lo = (x - hi.astype(np.float32)).astype(f8)
    return hi, lo


def kernel(**inputs) -> np.ndarray:
    hs = np.asarray(inputs["hidden_states"], np.float32)
    cos = np.asarray(inputs["cos"], np.float32)
    sin = np.asarray(inputs["sin"], np.float32)
    mask = np.asarray(inputs["attention_mask"], np.float32)
    Wq = np.asarray(inputs["Wq"], np.float32)
    Wk = np.asarray(inputs["Wk"], np.float32)
    Wv = np.asarray(inputs["Wv"], np.float32)
    Wo = np.asarray(inputs["Wo"], np.float32)

    m = mask.reshape(S, S)
    tril = np.tril(np.ones((S, S), dtype=bool))
    causal_ref = np.where(tril, np.float32(0.0), np.float32(NEG))
    if np.array_equal(m, causal_ref):
        causal = True
    elif not m.any():
        causal = False
    else:
        return _numpy_reference(hs, cos, sin, mask, Wq, Wk, Wv, Wo)

    nc = _get_nc(causal)
    consts = _host_consts()
    chan_half = (np.arange(64) // 2)

    bf = ml_dtypes.bfloat16
    in_maps = []
    for core in range(8):
        b, t = core // TP, core % TP
        # hidden states, swizzled [p, c, s], fp8 hi/lo
        hsw = np.ascontiguousarray(
            hs[b].T.reshape(8, 128, S).transpose(1, 0, 2))
        h_hi, h_lo = _split8(hsw)
        h_b = np.ascontiguousarray(np.concatenate([h_hi, h_lo], axis=1))

        # Wq shard [1024, 256] -> [p, pp, hl, dc, n]
        wq_s = Wq[:, t * 256:(t + 1) * 256] * WSCL
        wq_sw = wq_s.reshape(8, 128, 256).transpose(1, 0, 2)  # [p, dc, 256]
        wq_pp = np.stack([wq_sw[:, :, 0:128], wq_sw[:, :, 128:256]],
                         axis=1)                              # [p, pp, dc, n]
        wq_hi, wq_lo = _split8(wq_pp)
        wq8 = np.ascontiguousarray(
            np.stack([wq_hi, wq_lo], axis=2)).reshape(128, 2, 2, 1024)

        # Wk|Wv shard [1024, 128] -> [p, hl, dc, n]
        wkv_s = np.concatenate([Wk[:, t * 64:(t + 1) * 64],
                                Wv[:, t * 64:(t + 1) * 64]], axis=1) * WSCL
        wkv_sw = wkv_s.reshape(8, 128, 128).transpose(1, 0, 2)
        wkv_hi, wkv_lo = _split8(wkv_sw)
        wkv8 = np.ascontiguousarray(
            np.stack([wkv_hi, wkv_lo], axis=1)).reshape(128, 2, 1024)

        # rope tables: [64, S] expanded from half tables, / WSCL, dup to 128
        cs64v = cos[b].T[chan_half, :] / WSCL
        sn64v = sin[b].T[chan_half, :] / WSCL
        cs2 = np.ascontiguousarray(np.stack(
            [np.concatenate([cs64v, cs64v], axis=0),
             np.concatenate([sn64v, sn64v], axis=0)], axis=1)).astype(bf)

        wo_s = Wo[t * 256:(t + 1) * 256] / WSCL
        # ctxT channel order per chunk: c0 = [h0|h2], c1 = [h1|h3]
        wo_p = np.ascontiguousarray(
            np.concatenate([wo_s[0:64], wo_s[128:192],
                            wo_s[64:128], wo_s[192:256]], axis=0)).astype(bf)
        in_maps.append({
            "htb": h_b, "wq8": wq8, "wkv8": wkv8,
            "cs2": cs2, "wo": wo_p,
            "consts": consts.astype(bf),
        })

    res = run_bass_kernel_spmd(nc, in_maps, core_ids=list(range(8)))
    out = np.zeros((B, S, D), np.float32)
    for core in range(8):
        out[core // TP] += res.results[core]["out"].astype(np.float32)
    return out
